# revision 28
# baseline (speedup 1.0000x reference)
"""Trainium2 Bass kernel for nn_AttentionBlock (B=2, S=4096, HID=256, 8 heads).

Sharding: 8 cores = 2 batches x 4 query-chunks of 1024 queries.
Each core redundantly computes full K/V projections for its batch, then
attention for its 1024 queries over all 8 heads, then the output projection.
Host gathers by concatenation (no cross-core reduction needed).

Mask compaction: the (b,1,S) key mask zeroes whole keys for every query and
head, so masked keys are dropped up front. The host computes the surviving
key indices (padded to a multiple of 512; padding slots carry maskbias -1e4
so they contribute exactly zero). The device pulls the surviving K/V rows
with dma_gather(transpose=True) over a host-merged fp16 [S, 512] K||V
tensor: one SWDGE op per 512 keys that gathers AND transposes, landing
kvT [c, keys] tiles directly (no PE transposes, no evict copies).

Host-side prep: qT pre-transposed fp16 [256, 1024]; weights cast fp16;
maskbias {0,-1e4} fp32 precomputed; out-proj bias replicated [128,256];
gather indices int16 in the 16-partition wrapped layout.

Device pipeline per core (fp16 matmul datapath, fp32 PSUM accumulation):
  - Scores transposed: sT[k, q] via 4-way row-strip K=32 fp16 matmuls.
  - exp on ACT with per-partition mask bias and 1/sqrt(32) folded in.
  - PV fp16 matmuls col-packed in pairs with a ones-augmented V column per
    head, so the softmax denominator accumulates alongside weightedT.
  - Normalization: weightedT evicted to SBUF (ACT), denominator rows
    DMA-packed into one [8,512] tile, one reciprocal_approx_fast, K=8
    selector matmuls broadcast both head reciprocals per tile, one
    full-tile DVE multiply. Emission deferred past the next qc's first
    head group so the PE never stalls on it.
  - Output projection from stacked weightedT against zero-padded Wo;
    bias added on DVE during PSUM eviction.
"""

import numpy as np

import concourse.bacc as bacc
import concourse.bass as bass
from concourse import mybir
from concourse.tile import TileContext
from concourse.bass_utils import run_bass_kernel_spmd

F32 = mybir.dt.float32
F16 = mybir.dt.float16
I16 = mybir.dt.int16
AF = mybir.ActivationFunctionType

HID = 256
HEADS = 8
DH = 32
SK = 4096
SQ = 1024  # queries per core
SCALE = 1.0 / np.sqrt(32.0)
NEG = -10000.0

_CACHE = {}


def _build_nc(nkc, n_full):
    """nkc = number of 128-key tiles after mask compaction (multiple of 4).
    n_full = number of those tiles with no padded keys (maskbias all zero)."""
    skc = nkc * 128
    nc = bacc.Bacc("TRN2", target_bir_lowering=False, debug=False, num_devices=8)

    qt_d = nc.dram_tensor("qt_in", [HID, SQ], F16, kind="ExternalInput").ap()
    kv_d = nc.dram_tensor("kv_in", [SK, 512], F16, kind="ExternalInput").ap()
    mb_d = nc.dram_tensor("maskbias_in", [128, nkc], F32, kind="ExternalInput").ap()
    i_d = nc.dram_tensor("idx_in", [128, skc // 16], I16, kind="ExternalInput").ap()
    wq_d = nc.dram_tensor("wq", [HID, HID], F16, kind="ExternalInput").ap()
    wk_d = nc.dram_tensor("wk", [HID, HID], F16, kind="ExternalInput").ap()
    wv_d = nc.dram_tensor("wv", [HID, HID], F16, kind="ExternalInput").ap()
    wo_d = nc.dram_tensor("wo_arr", [128, 1024], F16, kind="ExternalInput").ap()
    bq_d = nc.dram_tensor("bq2", [128, 2], F32, kind="ExternalInput").ap()
    bk_d = nc.dram_tensor("bk2", [128, 2], F32, kind="ExternalInput").ap()
    bo_d = nc.dram_tensor("bo_rep", [128, HID], F32, kind="ExternalInput").ap()
    sel_d = nc.dram_tensor("sel8_in", [4, 256], F16, kind="ExternalInput").ap()
    out_d = nc.dram_tensor("out", [SQ, HID], F32, kind="ExternalOutput").ap()

    from contextlib import ExitStack

    with TileContext(nc) as tc, ExitStack() as top:
        const = top.enter_context(tc.tile_pool(name="const", bufs=1))
        persist = top.enter_context(tc.tile_pool(name="persist", bufs=1))
        io_pool = top.enter_context(tc.tile_pool(name="io", bufs=3))
        pt_pool = top.enter_context(tc.tile_pool(name="pt", bufs=4))
        rc_pool = top.enter_context(tc.tile_pool(name="rc", bufs=2))
        osb_pool = top.enter_context(tc.tile_pool(name="osb", bufs=8))
        ob_pool = top.enter_context(tc.tile_pool(name="ob", bufs=4))

        gpsum = top.enter_context(tc.tile_pool(name="gpsum", bufs=2, space="PSUM"))
        st_pool = top.enter_context(tc.tile_pool(name="stp", bufs=2, space="PSUM"))
        wt_pool = top.enter_context(tc.tile_pool(name="wtp", bufs=2, space="PSUM"))

        # ---------------- constants ----------------
        # idx first on the sync queue (gathers gate everything); bulk
        # weights go on the Activation hwdge queue in parallel
        idx_sb = const.tile([128, skc // 16], I16, name="idx_sb")
        nc.sync.dma_start(idx_sb, i_d)

        # warm-up gather: absorbs the Q7/SWDGE first-op latency so the
        # first real gather issues promptly
        idx_warm = const.tile([128, 8], I16, name="idx_warm")
        nc.vector.memset(idx_warm, 0)
        warm = const.tile([128, 4, 128], F16, name="warm")
        nc.gpsimd.dma_gather(warm, kv_d, idx_warm, 128, 128, 512,
                             transpose=True)
        wq_hf = []
        wk_hf = []
        wv_hf = []
        for t in range(2):
            for nm, d_ap, lst in (("wq", wq_d, wq_hf), ("wk", wk_d, wk_hf),
                                  ("wv", wv_d, wv_hf)):
                wb = const.tile([128, 256], F16, name=f"{nm}_h{t}")
                eng = nc.sync if nm == "wk" else nc.scalar
                eng.dma_start(wb, d_ap[t * 128:(t + 1) * 128, :])
                lst.append(wb)
        wo_hf = const.tile([128, 1024], F16, name="wo_hf")
        nc.scalar.dma_start(wo_hf, wo_d)
        bq_sb = const.tile([128, 2], F32, name="bq_sb")
        nc.scalar.dma_start(bq_sb, bq_d)
        bk_sb = const.tile([128, 2], F32, name="bk_sb")
        nc.sync.dma_start(bk_sb, bk_d)
        bo_sb = const.tile([128, HID], F32, name="bo_sb")
        nc.scalar.dma_start(bo_sb, bo_d)

        maskbias = const.tile([128, nkc], F32, name="maskbias")
        nc.scalar.dma_start(maskbias, mb_d)

        # selectors for the K=4 denominator broadcast matmuls: sel4[jj] maps
        # drows row 2jj -> output partitions 0-31, row 2jj+1 -> 64-95
        sel_sb = const.tile([4, 256], F16, name="sel_sb")
        nc.scalar.dma_start(sel_sb, sel_d)
        sel4 = [sel_sb[:, 128 * u:128 * (u + 1)] for u in range(2)]

        # ---------------- persistent buffers ----------------
        qT_sb = [persist.tile([128, SQ], F16, name=f"qT_sb{g}") for g in range(2)]
        kT_ch = [[persist.tile([128, 512], F16, name=f"kT{g}_{c}")
                  for c in range(skc // 512)] for g in range(2)]
        # augmented V, one [128, 264] tile per ktile (ones in column 33h+32)
        vaug = [persist.tile([128, 264], F16, name=f"vaug{s}")
                for s in range(nkc)]
        for s in range(nkc):
            nc.vector.memset(vaug[s], 1.0)
        wtn_all = [persist.tile([128, 512], F16, name=f"wtn{i}")
                   for i in range(8)]

        # ---------------- phase A ----------------
        # Q: host-pre-transposed fp16 -> project directly
        qx = []
        for t in range(2):
            qq = io_pool.tile([128, SQ], F16, tag="qx", name="qx", bufs=2)
            nc.scalar.dma_start(qq, qt_d[t * 128:(t + 1) * 128, :])
            qx.append(qq)
        for sg in range(SQ // 512):
            for g in range(2):
                ps = gpsum.tile([128, 512], F32, tag="gp", name="ps")
                for t in range(2):
                    nc.tensor.matmul(
                        ps, wq_hf[t][:, g * 128:(g + 1) * 128],
                        qx[t][:, sg * 512:(sg + 1) * 512],
                        start=(t == 0), stop=(t == 1))
                nc.vector.tensor_scalar_add(
                    qT_sb[g][:, sg * 512:(sg + 1) * 512], ps, bq_sb[:, g:g + 1])

        # K/V: one gather+transpose per 256 keys -> kvT [c-block, j, key]
        # kvT[p, j, n] = kv[idx[n], j*128 + p]; j=0,1 are K, j=2,3 are V.
        # 256-key granularity keeps Q7 descriptor generation off the
        # critical path of the first score matmuls.
        def emit_kv_chunk(cch):
            kvTh = []
            for h in range(2):
                kv_t = io_pool.tile([128, 4, 256], F16, tag="kvth",
                                    name="kvTh", bufs=10)
                nc.gpsimd.dma_gather(
                    kv_t, kv_d, idx_sb[:, 32 * cch + 16 * h:
                                       32 * cch + 16 * (h + 1)],
                    256, 256, 512, transpose=True)
                kvTh.append(kv_t)
            for g in range(2):
                ps = gpsum.tile([128, 512], F32, tag="gp", name="ps")
                for h in range(2):
                    for t in range(2):
                        nc.tensor.matmul(
                            ps[:, h * 256:(h + 1) * 256],
                            wk_hf[t][:, g * 128:(g + 1) * 128],
                            kvTh[h][:, t, :],
                            start=(t == 0), stop=(t == 1))
                    nc.vector.tensor_scalar_add(
                        kT_ch[g][cch][:, h * 256:(h + 1) * 256],
                        ps[:, h * 256:(h + 1) * 256], bk_sb[:, g:g + 1])
            for j in range(4):
                s = cch * 4 + j
                vps = gpsum.tile([128, 512], F32, tag="gp", name="vps")[:, 0:256]
                for t in range(2):
                    nc.tensor.matmul(
                        vps,
                        kvTh[j // 2][:, 2 + t, (j % 2) * 128:(j % 2 + 1) * 128],
                        wv_hf[t],
                        start=(t == 0), stop=(t == 1))
                dst = vaug[s].rearrange("p (h e) -> p h e", e=33)[:, :, 0:DH]
                src = vps.rearrange("p (h e) -> p h e", e=DH)
                nc.vector.tensor_copy(dst, src)

        # ---------------- attention ----------------
        def emit_attn_block(qc, g, wts, kt0, kt1, carry):
            """Scores + exp + PV for kt in [kt0, kt1), software-pipelined:
            scores(kt+1) are emitted before exp/PV(kt) so the PE never
            waits on the ACT exp chain. carry = [kt, sts] not yet PV'd."""
            for kt in range(kt0, kt1):
                sts = emit_scores(qc, g, kt)
                if carry[0] is not None:
                    emit_exp_pv(qc, g, wts, *carry)
                carry[0], carry[1] = kt, sts

        def emit_scores(qc, g, kt):
            sts = []
            for jj in range(2):
                st = st_pool.tile([128, 1024], F32, tag="st", name="st")
                for j2 in range(2):
                    j = 2 * jj + j2
                    nc.tensor.matmul(
                        st[:, j2 * 512:(j2 + 1) * 512],
                        kT_ch[g][kt // 4][32 * j:32 * j + 32,
                                          (kt % 4) * 128:
                                          (kt % 4) * 128 + 128],
                        qT_sb[g][32 * j:32 * j + 32,
                                 qc * 512:(qc + 1) * 512],
                        start=True, stop=True,
                        tile_position=(32 * j, 0))
                sts.append(st)
            return sts

        def emit_exp_pv(qc, g, wts, kt, sts):
            for jj in range(2):
                ptile = pt_pool.tile([128, 1024], F16, tag="pt",
                                     name="ptile")
                if kt < n_full:
                    nc.scalar.activation(ptile, sts[jj], AF.Exp,
                                         scale=SCALE)
                else:
                    nc.scalar.activation(ptile, sts[jj], AF.Exp,
                                         bias=maskbias[:, kt:kt + 1],
                                         scale=SCALE)
                for j2 in range(2):
                    h = 4 * g + 2 * jj + j2
                    nc.tensor.matmul(
                        wts[jj][64 * j2:64 * j2 + 33, :],
                        vaug[kt][:, 33 * h:33 * h + 33],
                        ptile[:, j2 * 512:(j2 + 1) * 512],
                        start=(kt == 0), stop=(kt == nkc - 1),
                        tile_position=(0, 64 * j2),
                        skip_group_check=True)

        def emit_attn_tail(qc, g, wts, carry):
            """Flush the pipelined last kt, then evict weightedT to SBUF
            wcop tiles on DVE; DMA the denominator rows straight out of
            the PSUM accumulators (strided 2-partition DMA)."""
            emit_exp_pv(qc, g, wts, *carry)
            wcops = []
            for jj in range(2):
                wcop = osb_pool.tile([128, 512], F32, tag="wcop", name="wcop")
                nc.vector.tensor_copy(wcop, wts[jj])
                nc.sync.dma_start(drows[qc][g][2 * jj:2 * jj + 2, :],
                                  wcop[32:97:64, :])
                wcops.append(wcop)
            wcop_all[qc][g] = wcops

        def emit_attn(qc, g):
            wts = [wt_pool.tile([128, 512], F32, tag="wt", name=f"wt{jj}")
                   for jj in range(2)]
            carry = [None, None]
            emit_attn_block(qc, g, wts, 0, nkc, carry)
            emit_attn_tail(qc, g, wts, carry)

        def emit_norm(qc, g):
            """Reciprocal + broadcast + normalize for one (qc, head-group)."""
            rrec = rc_pool.tile([4, 512], F32, tag="rrec", name="rrec")
            nc.vector.reciprocal_approx_fast(out=rrec, in_=drows[qc][g])
            rrech = rc_pool.tile([4, 512], F16, tag="rrech", name="rrech")
            nc.vector.tensor_copy(rrech, rrec)
            for jj in range(2):
                u = 2 * g + jj
                bc = gpsum.tile([128, 512], F32, tag="gp", name="bc")
                nc.tensor.matmul(bc, sel4[jj], rrech, start=True, stop=True)
                wtn = wtn_all[4 * qc + u]
                nc.vector.tensor_mul(wtn, wcop_all[qc][g][jj], bc)

        def emit_outproj(qc):
            wtns = [wtn_all[4 * qc + u] for u in range(4)]
            for m in range(4):
                ops = gpsum.tile([128, 512], F32, tag="gp", name="ops")[:, 0:256]
                for p in range(4):
                    nc.tensor.matmul(
                        ops, wtns[p][:, m * 128:(m + 1) * 128],
                        wo_hf[:, p * 256:(p + 1) * 256],
                        start=(p == 0), stop=(p == 3),
                        skip_group_check=True)
                ob = ob_pool.tile([128, 256], F32, tag="ob", name="ob")
                nc.vector.tensor_add(ob, ops, bo_sb)
                nc.sync.dma_start(
                    out_d[qc * 512 + m * 128:qc * 512 + (m + 1) * 128, :],
                    ob)

        drows = [[rc_pool.tile([4, 512], F32, tag=f"drows{qc}_{g}",
                               name=f"drows{qc}_{g}") for g in range(2)]
                 for qc in range(2)]
        wcop_all = [[None, None], [None, None]]

        # attention (0,0) interleaved with phase-A K/V chunks so the PE
        # fills gather waits with ready score work
        wts00 = [wt_pool.tile([128, 512], F32, tag="wt", name=f"wt00_{jj}")
                 for jj in range(2)]
        carry00 = [None, None]
        for cch in range(skc // 512):
            emit_kv_chunk(cch)
            emit_attn_block(0, 0, wts00, cch * 4, cch * 4 + 4, carry00)
        emit_attn_tail(0, 0, wts00, carry00)

        emit_attn(0, 1)
        emit_norm(0, 0)
        emit_attn(1, 0)
        emit_norm(0, 1)
        emit_outproj(0)
        emit_attn(1, 1)
        emit_norm(1, 0)
        emit_norm(1, 1)
        emit_outproj(1)

    nc.finalize()
    return nc


def _get_nc(nkc, n_full):
    key = ("nc", nkc, n_full)
    if key not in _CACHE:
        _CACHE[key] = _build_nc(nkc, n_full)
    return _CACHE[key]


def kernel(query, key, value, mask, Wq, bq, Wk, bk, Wv, bv, Wo, bo,
           _trace=False):
    query = np.asarray(query, np.float32)
    key = np.asarray(key, np.float32)
    value = np.asarray(value, np.float32)
    mask = np.asarray(mask, np.int32)
    Wq = np.ascontiguousarray(np.asarray(Wq, np.float16))
    Wk = np.ascontiguousarray(np.asarray(Wk, np.float16))
    Wv = np.ascontiguousarray(np.asarray(Wv, np.float16))
    Wo32 = np.asarray(Wo, np.float32)
    bq = np.asarray(bq, np.float32)
    bk = np.asarray(bk, np.float32)
    bv = np.asarray(bv, np.float32)
    bo = np.asarray(bo, np.float32)

    # mask compaction: indices of surviving keys per batch, padded to a
    # multiple of 512 with a duplicate (masked-out) index
    idxs = [np.nonzero(mask[b, 0])[0].astype(np.int32) for b in range(2)]
    nk_max = max(len(ix) for ix in idxs)
    nk_max = max(nk_max, 1)
    skc = ((nk_max + 511) // 512) * 512
    nkc = skc // 128
    # tiles [0, n_full) contain no padded keys on ANY core (bias-free exp)
    n_full = min(len(ix) for ix in idxs) // 128

    nc = _get_nc(nkc, n_full)

    wo_arr = np.zeros((128, 4, 256), np.float32)
    for p in range(4):
        wo_arr[0:32, p] = Wo32[64 * p:64 * p + 32]
        wo_arr[64:96, p] = Wo32[64 * p + 32:64 * p + 64]
    wo_arr = np.ascontiguousarray(wo_arr.reshape(128, 1024).astype(np.float16))
    bq2 = np.ascontiguousarray(bq.reshape(2, 128).T)
    bk2 = np.ascontiguousarray(bk.reshape(2, 128).T)
    bo_rep = np.ascontiguousarray(
        np.broadcast_to((bv @ Wo32 + bo).reshape(1, 256), (128, 256))
        .astype(np.float32))
    sel8_np = np.zeros((4, 2, 128), np.float16)
    for u in range(2):
        sel8_np[2 * u, u, 0:DH] = 1.0
        sel8_np[2 * u + 1, u, 64:64 + DH] = 1.0
    sel8_np = np.ascontiguousarray(sel8_np.reshape(4, 256))

    kv = [np.ascontiguousarray(
        np.concatenate([key[b], value[b]], axis=1).astype(np.float16))
        for b in range(2)]
    qt = [[np.ascontiguousarray(
        query[b, qi * SQ:(qi + 1) * SQ].T.astype(np.float16))
        for qi in range(4)] for b in range(2)]

    in_maps = []
    for c in range(8):
        b, qi = divmod(c, 4)
        ix = idxs[b]
        nk = len(ix)
        ix_pad = np.concatenate(
            [ix, np.full(skc - nk, ix[0] if nk else 0, np.int32)])
        mcomp = np.where(np.arange(skc) < nk, 0.0, NEG).astype(np.float32)
        mb = np.ascontiguousarray(mcomp.reshape(nkc, 128).T)
        # 16-partition wrapped int16 layout (idx n at [n % 16, n // 16]),
        # replicated into each of the 8 GpSimd core partition groups
        i16 = np.tile(ix_pad.astype(np.int16).reshape(skc // 16, 16).T, (8, 1))
        in_maps.append({
            "qt_in": qt[b][qi],
            "kv_in": kv[b],
            "maskbias_in": mb,
            "idx_in": np.ascontiguousarray(i16),
            "wq": Wq, "wk": Wk, "wv": Wv, "wo_arr": wo_arr,
            "bq2": bq2, "bk2": bk2, "bo_rep": bo_rep, "sel8_in": sel8_np,
        })

    res = run_bass_kernel_spmd(nc, in_maps, core_ids=list(range(8)),
                               trace=_trace)
    if _trace:
        _CACHE["last_result"] = res

    out = np.empty((2, 4096, HID), np.float32)
    for c in range(8):
        b, qi = divmod(c, 4)
        out[b, qi * SQ:(qi + 1) * SQ] = res.results[c]["out"]
    return out


# revision 29
# speedup vs baseline: 1.1052x; 1.1052x over previous
"""Trainium2 Bass kernel for nn_AttentionBlock (B=2, S=4096, HID=256, 8 heads).

Sharding: 8 cores = 2 batches x 4 query-chunks of 1024 queries.
Each core redundantly computes full K/V projections for its batch, then
attention for its 1024 queries over all 8 heads, then the output projection.
Host gathers by concatenation (no cross-core reduction needed).

Mask compaction: the (b,1,S) key mask zeroes whole keys for every query and
head, so masked keys are dropped up front. The host computes the surviving
key indices (padded to a multiple of 512; padding slots carry maskbias -1e4
so they contribute exactly zero). The device pulls the surviving K/V rows
with dma_gather(transpose=True) over a host-merged fp16 [S, 512] K||V
tensor: one SWDGE op per 512 keys that gathers AND transposes, landing
kvT [c, keys] tiles directly (no PE transposes, no evict copies).

Host-side prep: qT pre-transposed fp16 [256, 1024]; weights cast fp16;
maskbias {0,-1e4} fp32 precomputed; out-proj bias replicated [128,256];
gather indices int16 in the 16-partition wrapped layout.

Device pipeline per core (fp16 matmul datapath, fp32 PSUM accumulation):
  - Scores transposed: sT[k, q] via 4-way row-strip K=32 fp16 matmuls.
  - exp on ACT with per-partition mask bias and 1/sqrt(32) folded in.
  - PV fp16 matmuls col-packed in pairs with a ones-augmented V column per
    head, so the softmax denominator accumulates alongside weightedT.
  - Normalization: weightedT evicted to SBUF (ACT), denominator rows
    DMA-packed into one [8,512] tile, one reciprocal_approx_fast, K=8
    selector matmuls broadcast both head reciprocals per tile, one
    full-tile DVE multiply. Emission deferred past the next qc's first
    head group so the PE never stalls on it.
  - Output projection from stacked weightedT against zero-padded Wo;
    bias added on DVE during PSUM eviction.
"""

import numpy as np

import concourse.bacc as bacc
import concourse.bass as bass
from concourse import mybir
from concourse.tile import TileContext
from concourse.bass_utils import run_bass_kernel_spmd

F32 = mybir.dt.float32
F16 = mybir.dt.float16
I16 = mybir.dt.int16
AF = mybir.ActivationFunctionType

HID = 256
HEADS = 8
DH = 32
SK = 4096
SQ = 1024  # queries per core
SCALE = 1.0 / np.sqrt(32.0)
NEG = -10000.0

_CACHE = {}


def _build_nc(nkc, n_full):
    """nkc = number of 128-key tiles after mask compaction (multiple of 4).
    n_full = number of those tiles with no padded keys (maskbias all zero)."""
    skc = nkc * 128
    nc = bacc.Bacc("TRN2", target_bir_lowering=False, debug=False, num_devices=8)

    qt_d = nc.dram_tensor("qt_in", [HID, SQ], F16, kind="ExternalInput").ap()
    kvt_d = nc.dram_tensor("kvt_in", [512, skc], F16, kind="ExternalInput").ap()
    mb_d = nc.dram_tensor("maskbias_in", [128, nkc], F32, kind="ExternalInput").ap()
    wq_d = nc.dram_tensor("wq", [HID, HID], F16, kind="ExternalInput").ap()
    wk_d = nc.dram_tensor("wk", [HID, HID], F16, kind="ExternalInput").ap()
    wv_d = nc.dram_tensor("wv", [HID, HID], F16, kind="ExternalInput").ap()
    wo_d = nc.dram_tensor("wo_arr", [128, 1024], F16, kind="ExternalInput").ap()
    bq_d = nc.dram_tensor("bq2", [128, 2], F32, kind="ExternalInput").ap()
    bk_d = nc.dram_tensor("bk2", [128, 2], F32, kind="ExternalInput").ap()
    bo_d = nc.dram_tensor("bo_rep", [128, HID], F32, kind="ExternalInput").ap()
    sel_d = nc.dram_tensor("sel8_in", [4, 256], F16, kind="ExternalInput").ap()
    out_d = nc.dram_tensor("out", [SQ, HID], F32, kind="ExternalOutput").ap()

    from contextlib import ExitStack

    with TileContext(nc) as tc, ExitStack() as top:
        const = top.enter_context(tc.tile_pool(name="const", bufs=1))
        persist = top.enter_context(tc.tile_pool(name="persist", bufs=1))
        io_pool = top.enter_context(tc.tile_pool(name="io", bufs=3))
        pt_pool = top.enter_context(tc.tile_pool(name="pt", bufs=4))
        rc_pool = top.enter_context(tc.tile_pool(name="rc", bufs=2))
        osb_pool = top.enter_context(tc.tile_pool(name="osb", bufs=8))
        ob_pool = top.enter_context(tc.tile_pool(name="ob", bufs=4))

        gpsum = top.enter_context(tc.tile_pool(name="gpsum", bufs=2, space="PSUM"))
        st_pool = top.enter_context(tc.tile_pool(name="stp", bufs=2, space="PSUM"))
        wt_pool = top.enter_context(tc.tile_pool(name="wtp", bufs=2, space="PSUM"))

        # ---------------- constants ----------------
        # bulk weights go on the Activation hwdge queue; the sync queue
        # is reserved for the kvT tiles that gate attention start
        wq_hf = []
        wk_hf = []
        wv_hf = []
        for t in range(2):
            for nm, d_ap, lst in (("wq", wq_d, wq_hf), ("wk", wk_d, wk_hf),
                                  ("wv", wv_d, wv_hf)):
                wb = const.tile([128, 256], F16, name=f"{nm}_h{t}")
                eng = nc.sync if nm == "wk" else nc.scalar
                eng.dma_start(wb, d_ap[t * 128:(t + 1) * 128, :])
                lst.append(wb)
        wo_hf = const.tile([128, 1024], F16, name="wo_hf")
        nc.scalar.dma_start(wo_hf, wo_d)
        bq_sb = const.tile([128, 2], F32, name="bq_sb")
        nc.scalar.dma_start(bq_sb, bq_d)
        bk_sb = const.tile([128, 2], F32, name="bk_sb")
        nc.sync.dma_start(bk_sb, bk_d)
        bo_sb = const.tile([128, HID], F32, name="bo_sb")
        nc.scalar.dma_start(bo_sb, bo_d)

        maskbias = const.tile([128, nkc], F32, name="maskbias")
        nc.scalar.dma_start(maskbias, mb_d)

        # selectors for the K=4 denominator broadcast matmuls: sel4[jj] maps
        # drows row 2jj -> output partitions 0-31, row 2jj+1 -> 64-95
        sel_sb = const.tile([4, 256], F16, name="sel_sb")
        nc.scalar.dma_start(sel_sb, sel_d)
        sel4 = [sel_sb[:, 128 * u:128 * (u + 1)] for u in range(2)]

        # ---------------- persistent buffers ----------------
        qT_sb = [persist.tile([128, SQ], F16, name=f"qT_sb{g}") for g in range(2)]
        kT_ch = [[persist.tile([128, 512], F16, name=f"kT{g}_{c}")
                  for c in range(skc // 512)] for g in range(2)]
        # augmented V, one [128, 264] tile per ktile (ones in column 33h+32)
        vaug = [persist.tile([128, 264], F16, name=f"vaug{s}")
                for s in range(nkc)]
        for s in range(nkc):
            nc.vector.memset(vaug[s], 1.0)
        wtn_all = [persist.tile([128, 512], F16, name=f"wtn{i}")
                   for i in range(8)]

        # ---------------- phase A ----------------
        # Q: host-pre-transposed fp16 -> project directly
        qx = []
        for t in range(2):
            qq = io_pool.tile([128, SQ], F16, tag="qx", name="qx", bufs=2)
            nc.scalar.dma_start(qq, qt_d[t * 128:(t + 1) * 128, :])
            qx.append(qq)
        for sg in range(SQ // 512):
            for g in range(2):
                ps = gpsum.tile([128, 512], F32, tag="gp", name="ps")
                for t in range(2):
                    nc.tensor.matmul(
                        ps, wq_hf[t][:, g * 128:(g + 1) * 128],
                        qx[t][:, sg * 512:(sg + 1) * 512],
                        start=(t == 0), stop=(t == 1))
                nc.vector.tensor_scalar_add(
                    qT_sb[g][:, sg * 512:(sg + 1) * 512], ps, bq_sb[:, g:g + 1])

        # K/V: host-compacted, host-transposed kvT [512 c, skc keys] fp16.
        # Direct 2D DMA per c-block; K blocks on sync, V on scalar queue.
        # kvT[p, j, n] = kv_row(idx[n])[j*128 + p]; j=0,1 are K, j=2,3 are V
        def emit_kv_chunk(cch):
            kvT = io_pool.tile([128, 4, 512], F16, tag="kvt", name="kvT",
                               bufs=3)
            for j in range(4):
                eng = nc.sync if j < 2 else nc.scalar
                eng.dma_start(kvT[:, j, :],
                              kvt_d[128 * j:128 * (j + 1),
                                    512 * cch:512 * (cch + 1)])
            for g in range(2):
                ps = gpsum.tile([128, 512], F32, tag="gp", name="ps")
                for t in range(2):
                    nc.tensor.matmul(
                        ps, wk_hf[t][:, g * 128:(g + 1) * 128], kvT[:, t, :],
                        start=(t == 0), stop=(t == 1))
                nc.vector.tensor_scalar_add(
                    kT_ch[g][cch], ps, bk_sb[:, g:g + 1])
            for j in range(4):
                s = cch * 4 + j
                vps = gpsum.tile([128, 512], F32, tag="gp", name="vps")[:, 0:256]
                for t in range(2):
                    nc.tensor.matmul(
                        vps, kvT[:, 2 + t, j * 128:(j + 1) * 128], wv_hf[t],
                        start=(t == 0), stop=(t == 1))
                dst = vaug[s].rearrange("p (h e) -> p h e", e=33)[:, :, 0:DH]
                src = vps.rearrange("p (h e) -> p h e", e=DH)
                nc.vector.tensor_copy(dst, src)

        # ---------------- attention ----------------
        def emit_attn_block(qc, g, wts, kt0, kt1, carry):
            """Scores + exp + PV for kt in [kt0, kt1), software-pipelined:
            scores(kt+1) are emitted before exp/PV(kt) so the PE never
            waits on the ACT exp chain. carry = [kt, sts] not yet PV'd."""
            for kt in range(kt0, kt1):
                sts = emit_scores(qc, g, kt)
                if carry[0] is not None:
                    emit_exp_pv(qc, g, wts, *carry)
                carry[0], carry[1] = kt, sts

        def emit_scores(qc, g, kt):
            sts = []
            for jj in range(2):
                st = st_pool.tile([128, 1024], F32, tag="st", name="st")
                for j2 in range(2):
                    j = 2 * jj + j2
                    nc.tensor.matmul(
                        st[:, j2 * 512:(j2 + 1) * 512],
                        kT_ch[g][kt // 4][32 * j:32 * j + 32,
                                          (kt % 4) * 128:
                                          (kt % 4) * 128 + 128],
                        qT_sb[g][32 * j:32 * j + 32,
                                 qc * 512:(qc + 1) * 512],
                        start=True, stop=True,
                        tile_position=(32 * j, 0))
                sts.append(st)
            return sts

        def emit_exp_pv(qc, g, wts, kt, sts):
            for jj in range(2):
                ptile = pt_pool.tile([128, 1024], F16, tag="pt",
                                     name="ptile")
                if kt < n_full:
                    nc.scalar.activation(ptile, sts[jj], AF.Exp,
                                         scale=SCALE)
                else:
                    nc.scalar.activation(ptile, sts[jj], AF.Exp,
                                         bias=maskbias[:, kt:kt + 1],
                                         scale=SCALE)
                for j2 in range(2):
                    h = 4 * g + 2 * jj + j2
                    nc.tensor.matmul(
                        wts[jj][64 * j2:64 * j2 + 33, :],
                        vaug[kt][:, 33 * h:33 * h + 33],
                        ptile[:, j2 * 512:(j2 + 1) * 512],
                        start=(kt == 0), stop=(kt == nkc - 1),
                        tile_position=(0, 64 * j2),
                        skip_group_check=True)

        def emit_attn_tail(qc, g, wts, carry):
            """Flush the pipelined last kt, then evict weightedT to SBUF
            wcop tiles on DVE; DMA the denominator rows straight out of
            the PSUM accumulators (strided 2-partition DMA)."""
            emit_exp_pv(qc, g, wts, *carry)
            wcops = []
            for jj in range(2):
                wcop = osb_pool.tile([128, 512], F32, tag="wcop", name="wcop")
                nc.vector.tensor_copy(wcop, wts[jj])
                nc.sync.dma_start(drows[qc][g][2 * jj:2 * jj + 2, :],
                                  wcop[32:97:64, :])
                wcops.append(wcop)
            wcop_all[qc][g] = wcops

        def emit_attn(qc, g):
            wts = [wt_pool.tile([128, 512], F32, tag="wt", name=f"wt{jj}")
                   for jj in range(2)]
            carry = [None, None]
            emit_attn_block(qc, g, wts, 0, nkc, carry)
            emit_attn_tail(qc, g, wts, carry)

        def emit_norm(qc, g):
            """Reciprocal + broadcast + normalize for one (qc, head-group)."""
            rrec = rc_pool.tile([4, 512], F32, tag="rrec", name="rrec")
            nc.vector.reciprocal_approx_fast(out=rrec, in_=drows[qc][g])
            rrech = rc_pool.tile([4, 512], F16, tag="rrech", name="rrech")
            nc.vector.tensor_copy(rrech, rrec)
            for jj in range(2):
                u = 2 * g + jj
                bc = gpsum.tile([128, 512], F32, tag="gp", name="bc")
                nc.tensor.matmul(bc, sel4[jj], rrech, start=True, stop=True)
                wtn = wtn_all[4 * qc + u]
                nc.vector.tensor_mul(wtn, wcop_all[qc][g][jj], bc)

        def emit_outproj(qc):
            wtns = [wtn_all[4 * qc + u] for u in range(4)]
            for m in range(4):
                ops = gpsum.tile([128, 512], F32, tag="gp", name="ops")[:, 0:256]
                for p in range(4):
                    nc.tensor.matmul(
                        ops, wtns[p][:, m * 128:(m + 1) * 128],
                        wo_hf[:, p * 256:(p + 1) * 256],
                        start=(p == 0), stop=(p == 3),
                        skip_group_check=True)
                ob = ob_pool.tile([128, 256], F32, tag="ob", name="ob")
                nc.vector.tensor_add(ob, ops, bo_sb)
                nc.sync.dma_start(
                    out_d[qc * 512 + m * 128:qc * 512 + (m + 1) * 128, :],
                    ob)

        drows = [[rc_pool.tile([4, 512], F32, tag=f"drows{qc}_{g}",
                               name=f"drows{qc}_{g}") for g in range(2)]
                 for qc in range(2)]
        wcop_all = [[None, None], [None, None]]

        # attention (0,0) interleaved with phase-A K/V chunks so the PE
        # fills gather waits with ready score work
        wts00 = [wt_pool.tile([128, 512], F32, tag="wt", name=f"wt00_{jj}")
                 for jj in range(2)]
        carry00 = [None, None]
        for cch in range(skc // 512):
            emit_kv_chunk(cch)
            emit_attn_block(0, 0, wts00, cch * 4, cch * 4 + 4, carry00)
        emit_attn_tail(0, 0, wts00, carry00)

        emit_attn(0, 1)
        emit_norm(0, 0)
        emit_attn(1, 0)
        emit_norm(0, 1)
        emit_outproj(0)
        emit_attn(1, 1)
        emit_norm(1, 0)
        emit_norm(1, 1)
        emit_outproj(1)

    nc.finalize()
    return nc


def _get_nc(nkc, n_full):
    key = ("nc", nkc, n_full)
    if key not in _CACHE:
        _CACHE[key] = _build_nc(nkc, n_full)
    return _CACHE[key]


def kernel(query, key, value, mask, Wq, bq, Wk, bk, Wv, bv, Wo, bo,
           _trace=False):
    query = np.asarray(query, np.float32)
    key = np.asarray(key, np.float32)
    value = np.asarray(value, np.float32)
    mask = np.asarray(mask, np.int32)
    Wq = np.ascontiguousarray(np.asarray(Wq, np.float16))
    Wk = np.ascontiguousarray(np.asarray(Wk, np.float16))
    Wv = np.ascontiguousarray(np.asarray(Wv, np.float16))
    Wo32 = np.asarray(Wo, np.float32)
    bq = np.asarray(bq, np.float32)
    bk = np.asarray(bk, np.float32)
    bv = np.asarray(bv, np.float32)
    bo = np.asarray(bo, np.float32)

    # mask compaction: indices of surviving keys per batch, padded to a
    # multiple of 512 with a duplicate (masked-out) index
    idxs = [np.nonzero(mask[b, 0])[0].astype(np.int32) for b in range(2)]
    nk_max = max(len(ix) for ix in idxs)
    nk_max = max(nk_max, 1)
    skc = ((nk_max + 511) // 512) * 512
    nkc = skc // 128
    # tiles [0, n_full) contain no padded keys on ANY core (bias-free exp)
    n_full = min(len(ix) for ix in idxs) // 128

    nc = _get_nc(nkc, n_full)

    wo_arr = np.zeros((128, 4, 256), np.float32)
    for p in range(4):
        wo_arr[0:32, p] = Wo32[64 * p:64 * p + 32]
        wo_arr[64:96, p] = Wo32[64 * p + 32:64 * p + 64]
    wo_arr = np.ascontiguousarray(wo_arr.reshape(128, 1024).astype(np.float16))
    bq2 = np.ascontiguousarray(bq.reshape(2, 128).T)
    bk2 = np.ascontiguousarray(bk.reshape(2, 128).T)
    bo_rep = np.ascontiguousarray(
        np.broadcast_to((bv @ Wo32 + bo).reshape(1, 256), (128, 256))
        .astype(np.float32))
    sel8_np = np.zeros((4, 2, 128), np.float16)
    for u in range(2):
        sel8_np[2 * u, u, 0:DH] = 1.0
        sel8_np[2 * u + 1, u, 64:64 + DH] = 1.0
    sel8_np = np.ascontiguousarray(sel8_np.reshape(4, 256))

    kv = [np.concatenate([key[b], value[b]], axis=1).astype(np.float16)
          for b in range(2)]
    qt = [[np.ascontiguousarray(
        query[b, qi * SQ:(qi + 1) * SQ].T.astype(np.float16))
        for qi in range(4)] for b in range(2)]

    # host-side compaction + transpose: kvT [512 c, skc keys] per batch
    kvt = []
    mbs = []
    for b in range(2):
        ix = idxs[b]
        nk = len(ix)
        ix_pad = np.concatenate(
            [ix, np.full(skc - nk, ix[0] if nk else 0, np.int32)])
        kvt.append(np.ascontiguousarray(kv[b][ix_pad].T))
        mcomp = np.where(np.arange(skc) < nk, 0.0, NEG).astype(np.float32)
        mbs.append(np.ascontiguousarray(mcomp.reshape(nkc, 128).T))

    in_maps = []
    for c in range(8):
        b, qi = divmod(c, 4)
        in_maps.append({
            "qt_in": qt[b][qi],
            "kvt_in": kvt[b],
            "maskbias_in": mbs[b],
            "wq": Wq, "wk": Wk, "wv": Wv, "wo_arr": wo_arr,
            "bq2": bq2, "bk2": bk2, "bo_rep": bo_rep, "sel8_in": sel8_np,
        })

    res = run_bass_kernel_spmd(nc, in_maps, core_ids=list(range(8)),
                               trace=_trace)
    if _trace:
        _CACHE["last_result"] = res

    out = np.empty((2, 4096, HID), np.float32)
    for c in range(8):
        b, qi = divmod(c, 4)
        out[b, qi * SQ:(qi + 1) * SQ] = res.results[c]["out"]
    return out


# revision 33
# speedup vs baseline: 1.1269x; 1.0196x over previous
"""Trainium2 Bass kernel for nn_AttentionBlock (B=2, S=4096, HID=256, 8 heads).

Sharding: 8 cores = 2 batches x 4 query-chunks of 1024 queries.
Each core redundantly computes full K/V projections for its batch, then
attention for its 1024 queries over all 8 heads, then the output projection.
Host gathers by concatenation (no cross-core reduction needed).

Mask compaction: the (b,1,S) key mask zeroes whole keys for every query and
head, so masked keys are dropped up front. The host computes the surviving
key indices (padded to a multiple of 512; padding slots carry maskbias -1e4
so they contribute exactly zero). The device pulls the surviving K/V rows
with dma_gather(transpose=True) over a host-merged fp16 [S, 512] K||V
tensor: one SWDGE op per 512 keys that gathers AND transposes, landing
kvT [c, keys] tiles directly (no PE transposes, no evict copies).

Host-side prep: qT pre-transposed fp16 [256, 1024]; weights cast fp16;
maskbias {0,-1e4} fp32 precomputed; out-proj bias replicated [128,256];
gather indices int16 in the 16-partition wrapped layout.

Device pipeline per core (fp16 matmul datapath, fp32 PSUM accumulation):
  - Scores transposed: sT[k, q] via 4-way row-strip K=32 fp16 matmuls.
  - exp on ACT with per-partition mask bias and 1/sqrt(32) folded in.
  - PV fp16 matmuls col-packed in pairs with a ones-augmented V column per
    head, so the softmax denominator accumulates alongside weightedT.
  - Normalization: weightedT evicted to SBUF (ACT), denominator rows
    DMA-packed into one [8,512] tile, one reciprocal_approx_fast, K=8
    selector matmuls broadcast both head reciprocals per tile, one
    full-tile DVE multiply. Emission deferred past the next qc's first
    head group so the PE never stalls on it.
  - Output projection from stacked weightedT against zero-padded Wo;
    bias added on DVE during PSUM eviction.
"""

import numpy as np

import concourse.bacc as bacc
import concourse.bass as bass
from concourse import mybir
from concourse.tile import TileContext
from concourse.bass_utils import run_bass_kernel_spmd

F32 = mybir.dt.float32
F32R = mybir.dt.float32r
F16 = mybir.dt.float16
I16 = mybir.dt.int16
AF = mybir.ActivationFunctionType

HID = 256
HEADS = 8
DH = 32
SK = 4096
SQ = 1024  # queries per core
SCALE = 1.0 / np.sqrt(32.0)
NEG = -10000.0

_CACHE = {}


def _build_nc(nkc, n_full):
    """nkc = number of 128-key tiles after mask compaction (multiple of 4).
    n_full = number of those tiles with no padded keys (maskbias all zero)."""
    skc = nkc * 128
    nc = bacc.Bacc("TRN2", target_bir_lowering=False, debug=False, num_devices=8)

    qt_d = nc.dram_tensor("qt_in", [128, 2 * SQ], F16, kind="ExternalInput").ap()
    kvt_d = nc.dram_tensor("kvt_in", [128, 4, skc], F16, kind="ExternalInput").ap()
    # packed fp16 consts: wq|wk|wv (6 x 256) then wo_arr (1024) then sel (256)
    ch_d = nc.dram_tensor("ch_in", [128, 2816], F16, kind="ExternalInput").ap()
    # packed fp32 consts: bq2 (2) | bk2 (2) | bo_rep (256) | maskbias (nkc)
    # | sel (2 x 128, rows 0-3 only)
    cf_d = nc.dram_tensor("cf_in", [128, 516 + nkc], F32, kind="ExternalInput").ap()
    out_d = nc.dram_tensor("out", [SQ, HID], F32, kind="ExternalOutput").ap()

    from contextlib import ExitStack

    with TileContext(nc) as tc, ExitStack() as top:
        const = top.enter_context(tc.tile_pool(name="const", bufs=1))
        persist = top.enter_context(tc.tile_pool(name="persist", bufs=1))
        io_pool = top.enter_context(tc.tile_pool(name="io", bufs=3))
        pt_pool = top.enter_context(tc.tile_pool(name="pt", bufs=4))
        rc_pool = top.enter_context(tc.tile_pool(name="rc", bufs=2))
        osb_pool = top.enter_context(tc.tile_pool(name="osb", bufs=8))
        ob_pool = top.enter_context(tc.tile_pool(name="ob", bufs=4))

        gpsum = top.enter_context(tc.tile_pool(name="gpsum", bufs=2, space="PSUM"))
        st_pool = top.enter_context(tc.tile_pool(name="stp", bufs=2, space="PSUM"))
        wt_pool = top.enter_context(tc.tile_pool(name="wtp", bufs=2, space="PSUM"))

        # ---------------- constants ----------------
        # three packed const DMAs on the Activation hwdge queue (the sync
        # queue carries the kvT chunks that gate attention start)
        qx_sb = const.tile([128, 2 * SQ], F16, name="qx_sb")
        nc.scalar.dma_start(qx_sb, qt_d)
        ch_sb = const.tile([128, 2816], F16, name="ch_sb")
        nc.scalar.dma_start(ch_sb[:, 0:1536], ch_d[:, 0:1536])
        nc.scalar.dma_start(ch_sb[:, 1536:2816], ch_d[:, 1536:2816])
        cf_sb = const.tile([128, 516 + nkc], F32, name="cf_sb")
        nc.scalar.dma_start(cf_sb, cf_d)

        wq_hf = [ch_sb[:, 0:256], ch_sb[:, 256:512]]
        wk_hf = [ch_sb[:, 512:768], ch_sb[:, 768:1024]]
        wv_hf = [ch_sb[:, 1024:1280], ch_sb[:, 1280:1536]]
        wo_hf = ch_sb[:, 1536:2560]
        sel4 = [ch_sb[0:4, 2560 + 128 * u:2560 + 128 * (u + 1)]
                for u in range(2)]
        bq_sb = cf_sb[:, 0:2]
        bk_sb = cf_sb[:, 2:4]
        bo_sb = cf_sb[:, 4:260]
        maskbias = cf_sb[:, 260:260 + nkc]

        # ---------------- persistent buffers ----------------
        qT_sb = [persist.tile([128, SQ], F16, name=f"qT_sb{g}") for g in range(2)]
        kT_ch = [[persist.tile([128, 512], F16, name=f"kT{g}_{c}")
                  for c in range(skc // 512)] for g in range(2)]
        # augmented V, one [128, 264] tile per ktile (ones in column 33h+32)
        vaug = [persist.tile([128, 264], F16, name=f"vaug{s}")
                for s in range(nkc)]
        for s in range(nkc):
            nc.vector.memset(vaug[s], 1.0)
        wtn_all = [persist.tile([128, 512], F16, name=f"wtn{i}")
                   for i in range(8)]

        # ---------------- phase A ----------------
        # Q: host-pre-transposed fp16 (both c-halves packed side by side)
        qx = [qx_sb[:, t * SQ:(t + 1) * SQ] for t in range(2)]
        for sg in range(SQ // 512):
            for g in range(2):
                ps = gpsum.tile([128, 512], F32, tag="gp", name="ps")
                for t in range(2):
                    nc.tensor.matmul(
                        ps, wq_hf[t][:, g * 128:(g + 1) * 128],
                        qx[t][:, sg * 512:(sg + 1) * 512],
                        start=(t == 0), stop=(t == 1))
                nc.vector.tensor_scalar_add(
                    qT_sb[g][:, sg * 512:(sg + 1) * 512], ps, bq_sb[:, g:g + 1])

        # K/V: host-compacted, host-transposed kvT [512 c, skc keys] fp16.
        # Direct 2D DMA per c-block; K blocks on sync, V on scalar queue.
        # kvT[p, j, n] = kv_row(idx[n])[j*128 + p]; j=0,1 are K, j=2,3 are V
        def emit_kv_chunk(cch):
            kvT = io_pool.tile([128, 4, 512], F16, tag="kvt", name="kvT",
                               bufs=3)
            nc.sync.dma_start(kvT, kvt_d[:, :, 512 * cch:512 * (cch + 1)])
            for g in range(2):
                ps = gpsum.tile([128, 512], F32, tag="gp", name="ps")
                for t in range(2):
                    nc.tensor.matmul(
                        ps, wk_hf[t][:, g * 128:(g + 1) * 128], kvT[:, t, :],
                        start=(t == 0), stop=(t == 1))
                nc.vector.tensor_scalar_add(
                    kT_ch[g][cch], ps, bk_sb[:, g:g + 1])
            for j in range(4):
                s = cch * 4 + j
                vps = gpsum.tile([128, 512], F32, tag="gp", name="vps")[:, 0:256]
                for t in range(2):
                    nc.tensor.matmul(
                        vps, kvT[:, 2 + t, j * 128:(j + 1) * 128], wv_hf[t],
                        start=(t == 0), stop=(t == 1))
                dst = vaug[s].rearrange("p (h e) -> p h e", e=33)[:, :, 0:DH]
                src = vps.rearrange("p (h e) -> p h e", e=DH)
                nc.vector.tensor_copy(dst, src)

        # ---------------- attention ----------------
        def emit_attn_block(qc, g, wts, kt0, kt1, carry):
            """Scores + exp + PV for kt in [kt0, kt1), software-pipelined:
            scores(kt+1) are emitted before exp/PV(kt) so the PE never
            waits on the ACT exp chain. carry = [kt, sts] not yet PV'd."""
            for kt in range(kt0, kt1):
                sts = emit_scores(qc, g, kt)
                if carry[0] is not None:
                    emit_exp_pv(qc, g, wts, *carry)
                carry[0], carry[1] = kt, sts

        def emit_scores(qc, g, kt):
            sts = []
            for jj in range(2):
                st = st_pool.tile([128, 1024], F32, tag="st", name="st")
                for j2 in range(2):
                    j = 2 * jj + j2
                    nc.tensor.matmul(
                        st[:, j2 * 512:(j2 + 1) * 512],
                        kT_ch[g][kt // 4][32 * j:32 * j + 32,
                                          (kt % 4) * 128:
                                          (kt % 4) * 128 + 128],
                        qT_sb[g][32 * j:32 * j + 32,
                                 qc * 512:(qc + 1) * 512],
                        start=True, stop=True,
                        tile_position=(32 * j, 0))
                sts.append(st)
            return sts

        def emit_exp_pv(qc, g, wts, kt, sts):
            for jj in range(2):
                ptile = pt_pool.tile([128, 1024], F16, tag="pt",
                                     name="ptile")
                if kt < n_full:
                    nc.scalar.activation(ptile, sts[jj], AF.Exp,
                                         scale=SCALE)
                else:
                    nc.scalar.activation(ptile, sts[jj], AF.Exp,
                                         bias=maskbias[:, kt:kt + 1],
                                         scale=SCALE)
                for j2 in range(2):
                    h = 4 * g + 2 * jj + j2
                    nc.tensor.matmul(
                        wts[jj][64 * j2:64 * j2 + 33, :],
                        vaug[kt][:, 33 * h:33 * h + 33],
                        ptile[:, j2 * 512:(j2 + 1) * 512],
                        start=(kt == 0), stop=(kt == nkc - 1),
                        tile_position=(0, 64 * j2),
                        skip_group_check=True)

        def emit_attn_tail(qc, g, wts, carry):
            """Flush the pipelined last kt, then evict weightedT to SBUF
            wcop tiles on DVE; DMA the denominator rows straight out of
            the PSUM accumulators (strided 2-partition DMA)."""
            emit_exp_pv(qc, g, wts, *carry)
            wcops = []
            for jj in range(2):
                wcop = osb_pool.tile([128, 512], F32, tag="wcop", name="wcop")
                nc.vector.tensor_copy(wcop, wts[jj])
                nc.sync.dma_start(drows[qc][g][2 * jj:2 * jj + 2, :],
                                  wcop[32:97:64, :])
                wcops.append(wcop)
            wcop_all[qc][g] = wcops

        def emit_attn(qc, g):
            wts = [wt_pool.tile([128, 512], F32, tag="wt", name=f"wt{jj}")
                   for jj in range(2)]
            carry = [None, None]
            emit_attn_block(qc, g, wts, 0, nkc, carry)
            emit_attn_tail(qc, g, wts, carry)

        def emit_norm(qc, g):
            """Reciprocal + broadcast (fp32r matmul) + normalize for one
            (qc, head-group)."""
            rrec = rc_pool.tile([4, 512], F32, tag="rrec", name="rrec")
            nc.vector.reciprocal_approx_fast(out=rrec, in_=drows[qc][g])
            rrech = rc_pool.tile([4, 512], F16, tag="rrech", name="rrech")
            nc.vector.tensor_copy(rrech, rrec)
            for jj in range(2):
                u = 2 * g + jj
                bc = gpsum.tile([128, 512], F32, tag="gp", name="bc")
                nc.tensor.matmul(bc, sel4[jj], rrech, start=True, stop=True)
                wtn = wtn_all[4 * qc + u]
                nc.vector.tensor_mul(wtn, wcop_all[qc][g][jj], bc)

        def emit_outproj(qc):
            wtns = [wtn_all[4 * qc + u] for u in range(4)]
            for m in range(4):
                ops = gpsum.tile([128, 512], F32, tag="gp", name="ops")[:, 0:256]
                for p in range(4):
                    nc.tensor.matmul(
                        ops, wtns[p][:, m * 128:(m + 1) * 128],
                        wo_hf[:, p * 256:(p + 1) * 256],
                        start=(p == 0), stop=(p == 3),
                        skip_group_check=True)
                ob = ob_pool.tile([128, 256], F32, tag="ob", name="ob")
                nc.vector.tensor_add(ob, ops, bo_sb)
                nc.sync.dma_start(
                    out_d[qc * 512 + m * 128:qc * 512 + (m + 1) * 128, :],
                    ob)

        drows = [[rc_pool.tile([4, 512], F32, tag=f"drows{qc}_{g}",
                               name=f"drows{qc}_{g}") for g in range(2)]
                 for qc in range(2)]
        wcop_all = [[None, None], [None, None]]

        # attention (0,0) interleaved with phase-A K/V chunks so the PE
        # fills gather waits with ready score work
        wts00 = [wt_pool.tile([128, 512], F32, tag="wt", name=f"wt00_{jj}")
                 for jj in range(2)]
        carry00 = [None, None]
        for cch in range(skc // 512):
            emit_kv_chunk(cch)
            emit_attn_block(0, 0, wts00, cch * 4, cch * 4 + 4, carry00)
        emit_attn_tail(0, 0, wts00, carry00)

        emit_attn(0, 1)
        emit_norm(0, 0)
        emit_attn(1, 0)
        emit_norm(0, 1)
        emit_outproj(0)
        emit_attn(1, 1)
        emit_norm(1, 0)
        emit_norm(1, 1)
        emit_outproj(1)

    nc.finalize()
    return nc


def _get_nc(nkc, n_full):
    key = ("nc", nkc, n_full)
    if key not in _CACHE:
        _CACHE[key] = _build_nc(nkc, n_full)
    return _CACHE[key]


def kernel(query, key, value, mask, Wq, bq, Wk, bk, Wv, bv, Wo, bo,
           _trace=False):
    query = np.asarray(query, np.float32)
    key = np.asarray(key, np.float32)
    value = np.asarray(value, np.float32)
    mask = np.asarray(mask, np.int32)
    Wq = np.ascontiguousarray(np.asarray(Wq, np.float16))
    Wk = np.ascontiguousarray(np.asarray(Wk, np.float16))
    Wv = np.ascontiguousarray(np.asarray(Wv, np.float16))
    Wo32 = np.asarray(Wo, np.float32)
    bq = np.asarray(bq, np.float32)
    bk = np.asarray(bk, np.float32)
    bv = np.asarray(bv, np.float32)
    bo = np.asarray(bo, np.float32)

    # mask compaction: indices of surviving keys per batch, padded to a
    # multiple of 512 with a duplicate (masked-out) index
    idxs = [np.nonzero(mask[b, 0])[0].astype(np.int32) for b in range(2)]
    nk_max = max(len(ix) for ix in idxs)
    nk_max = max(nk_max, 1)
    skc = ((nk_max + 511) // 512) * 512
    nkc = skc // 128
    # tiles [0, n_full) contain no padded keys on ANY core (bias-free exp)
    n_full = min(len(ix) for ix in idxs) // 128

    nc = _get_nc(nkc, n_full)

    wo_arr = np.zeros((128, 4, 256), np.float32)
    for p in range(4):
        wo_arr[0:32, p] = Wo32[64 * p:64 * p + 32]
        wo_arr[64:96, p] = Wo32[64 * p + 32:64 * p + 64]
    wo_arr = wo_arr.reshape(128, 1024).astype(np.float16)

    # packed fp16 consts: wq | wk | wv (rows t-major) then wo_arr
    ch = np.zeros((128, 2816), np.float16)
    for t in range(2):
        ch[:, 256 * t:256 * (t + 1)] = Wq[128 * t:128 * (t + 1)]
        ch[:, 512 + 256 * t:512 + 256 * (t + 1)] = Wk[128 * t:128 * (t + 1)]
        ch[:, 1024 + 256 * t:1024 + 256 * (t + 1)] = Wv[128 * t:128 * (t + 1)]
    ch[:, 1536:2560] = wo_arr
    sel_f = np.zeros((128, 2, 128), np.float16)
    for u in range(2):
        sel_f[2 * u, u, 0:DH] = 1.0
        sel_f[2 * u + 1, u, 64:64 + DH] = 1.0
    ch[:, 2560:2816] = sel_f.reshape(128, 256)
    ch = np.ascontiguousarray(ch)

    # packed fp32 consts per batch: bq2 | bk2 | bo_rep | maskbias | sel
    bq2 = bq.reshape(2, 128).T
    bk2 = bk.reshape(2, 128).T
    bo_rep = np.broadcast_to((bv @ Wo32 + bo).reshape(1, 256), (128, 256))

    kv = [np.concatenate([key[b], value[b]], axis=1).astype(np.float16)
          for b in range(2)]
    qt2 = []
    for b in range(2):
        row = []
        for qi in range(4):
            qT = query[b, qi * SQ:(qi + 1) * SQ].T.astype(np.float16)
            row.append(np.ascontiguousarray(
                np.concatenate([qT[0:128], qT[128:256]], axis=1)))
        qt2.append(row)

    # host-side compaction + transpose: kvt5 [128, 4, skc] per batch
    kvt = []
    cfs = []
    for b in range(2):
        ix = idxs[b]
        nk = len(ix)
        ix_pad = np.concatenate(
            [ix, np.full(skc - nk, ix[0] if nk else 0, np.int32)])
        kvT = kv[b][ix_pad].T  # [512, skc]
        kvt.append(np.ascontiguousarray(kvT.reshape(4, 128, skc)
                                        .transpose(1, 0, 2)))
        mcomp = np.where(np.arange(skc) < nk, 0.0, NEG).astype(np.float32)
        mb = mcomp.reshape(nkc, 128).T
        cf = np.zeros((128, 516 + nkc), np.float32)
        cf[:, 0:2] = bq2
        cf[:, 2:4] = bk2
        cf[:, 4:260] = bo_rep
        cf[:, 260:260 + nkc] = mb
        cfs.append(np.ascontiguousarray(cf))

    in_maps = []
    for c in range(8):
        b, qi = divmod(c, 4)
        in_maps.append({
            "qt_in": qt2[b][qi],
            "kvt_in": kvt[b],
            "ch_in": ch,
            "cf_in": cfs[b],
        })

    res = run_bass_kernel_spmd(nc, in_maps, core_ids=list(range(8)),
                               trace=_trace)
    if _trace:
        _CACHE["last_result"] = res

    out = np.empty((2, 4096, HID), np.float32)
    for c in range(8):
        b, qi = divmod(c, 4)
        out[b, qi * SQ:(qi + 1) * SQ] = res.results[c]["out"]
    return out


# revision 35
# speedup vs baseline: 1.1283x; 1.0012x over previous
"""Trainium2 Bass kernel for nn_AttentionBlock (B=2, S=4096, HID=256, 8 heads).

Sharding: 8 cores = 2 batches x 4 query-chunks of 1024 queries.
Each core redundantly computes full K/V projections for its batch, then
attention for its 1024 queries over all 8 heads, then the output projection.
Host gathers by concatenation (no cross-core reduction needed).

Mask compaction: the (b,1,S) key mask zeroes whole keys for every query and
head, so masked keys are dropped up front. The host computes the surviving
key indices (padded to a multiple of 512; padding slots carry maskbias -1e4
so they contribute exactly zero). The device pulls the surviving K/V rows
with dma_gather(transpose=True) over a host-merged fp16 [S, 512] K||V
tensor: one SWDGE op per 512 keys that gathers AND transposes, landing
kvT [c, keys] tiles directly (no PE transposes, no evict copies).

Host-side prep: qT pre-transposed fp16 [256, 1024]; weights cast fp16;
maskbias {0,-1e4} fp32 precomputed; out-proj bias replicated [128,256];
gather indices int16 in the 16-partition wrapped layout.

Device pipeline per core (fp16 matmul datapath, fp32 PSUM accumulation):
  - Scores transposed: sT[k, q] via 4-way row-strip K=32 fp16 matmuls.
  - exp on ACT with per-partition mask bias and 1/sqrt(32) folded in.
  - PV fp16 matmuls col-packed in pairs with a ones-augmented V column per
    head, so the softmax denominator accumulates alongside weightedT.
  - Normalization: weightedT evicted to SBUF (ACT), denominator rows
    DMA-packed into one [8,512] tile, one reciprocal_approx_fast, K=8
    selector matmuls broadcast both head reciprocals per tile, one
    full-tile DVE multiply. Emission deferred past the next qc's first
    head group so the PE never stalls on it.
  - Output projection from stacked weightedT against zero-padded Wo;
    bias added on DVE during PSUM eviction.
"""

import numpy as np

import concourse.bacc as bacc
import concourse.bass as bass
from concourse import mybir
from concourse.tile import TileContext
from concourse.bass_utils import run_bass_kernel_spmd

F32 = mybir.dt.float32
F32R = mybir.dt.float32r
F16 = mybir.dt.float16
I16 = mybir.dt.int16
AF = mybir.ActivationFunctionType

HID = 256
HEADS = 8
DH = 32
SK = 4096
SQ = 1024  # queries per core
SCALE = 1.0 / np.sqrt(32.0)
NEG = -10000.0

_CACHE = {}


def _build_nc(nkc, n_full):
    """nkc = number of 128-key tiles after mask compaction (multiple of 4).
    n_full = number of those tiles with no padded keys (maskbias all zero)."""
    skc = nkc * 128
    nc = bacc.Bacc("TRN2", target_bir_lowering=False, debug=False, num_devices=8)

    qt_d = nc.dram_tensor("qt_in", [128, 2 * SQ], F16, kind="ExternalInput").ap()
    kvt_d = nc.dram_tensor("kvt_in", [128, 4, skc], F16, kind="ExternalInput").ap()
    # packed fp16 consts: wq|wk|wv (6 x 256) then wo_arr (1024) then sel (256)
    ch_d = nc.dram_tensor("ch_in", [128, 2816], F16, kind="ExternalInput").ap()
    # packed fp32 consts: bq2 (2) | bk2 (2) | bo_rep (256) | maskbias (nkc)
    # | sel (2 x 128, rows 0-3 only)
    cf_d = nc.dram_tensor("cf_in", [128, 516 + nkc], F32, kind="ExternalInput").ap()
    out_d = nc.dram_tensor("out", [SQ, HID], F32, kind="ExternalOutput").ap()

    from contextlib import ExitStack

    with TileContext(nc) as tc, ExitStack() as top:
        const = top.enter_context(tc.tile_pool(name="const", bufs=1))
        persist = top.enter_context(tc.tile_pool(name="persist", bufs=1))
        io_pool = top.enter_context(tc.tile_pool(name="io", bufs=3))
        pt_pool = top.enter_context(tc.tile_pool(name="pt", bufs=4))
        rc_pool = top.enter_context(tc.tile_pool(name="rc", bufs=2))
        osb_pool = top.enter_context(tc.tile_pool(name="osb", bufs=8))
        ob_pool = top.enter_context(tc.tile_pool(name="ob", bufs=4))

        gpsum = top.enter_context(tc.tile_pool(name="gpsum", bufs=2, space="PSUM"))
        st_pool = top.enter_context(tc.tile_pool(name="stp", bufs=2, space="PSUM"))
        wt_pool = top.enter_context(tc.tile_pool(name="wtp", bufs=2, space="PSUM"))

        # ---------------- constants ----------------
        # three packed const DMAs on the Activation hwdge queue (the sync
        # queue carries the kvT chunks that gate attention start)
        qx_sb = const.tile([128, 2 * SQ], F16, name="qx_sb")
        nc.scalar.dma_start(qx_sb[:, 0:SQ], qt_d[:, 0:SQ])
        ch_sb = const.tile([128, 2816], F16, name="ch_sb")
        nc.scalar.dma_start(ch_sb[:, 0:1536], ch_d[:, 0:1536])
        nc.scalar.dma_start(ch_sb[:, 1536:2816], ch_d[:, 1536:2816])
        nc.scalar.dma_start(qx_sb[:, SQ:2 * SQ], qt_d[:, SQ:2 * SQ])
        cf_sb = const.tile([128, 516 + nkc], F32, name="cf_sb")
        nc.scalar.dma_start(cf_sb, cf_d)

        wq_hf = [ch_sb[:, 0:256], ch_sb[:, 256:512]]
        wk_hf = [ch_sb[:, 512:768], ch_sb[:, 768:1024]]
        wv_hf = [ch_sb[:, 1024:1280], ch_sb[:, 1280:1536]]
        wo_hf = ch_sb[:, 1536:2560]
        sel4 = [ch_sb[0:4, 2560 + 128 * u:2560 + 128 * (u + 1)]
                for u in range(2)]
        bq_sb = cf_sb[:, 0:2]
        bk_sb = cf_sb[:, 2:4]
        bo_sb = cf_sb[:, 4:260]
        maskbias = cf_sb[:, 260:260 + nkc]

        # ---------------- persistent buffers ----------------
        qT_sb = [persist.tile([128, SQ], F16, name=f"qT_sb{g}") for g in range(2)]
        kT_ch = [[persist.tile([128, 512], F16, name=f"kT{g}_{c}")
                  for c in range(skc // 512)] for g in range(2)]
        # augmented V, one [128, 264] tile per ktile (ones in column 33h+32)
        vaug = [persist.tile([128, 264], F16, name=f"vaug{s}")
                for s in range(nkc)]
        for s in range(nkc):
            nc.vector.memset(vaug[s], 1.0)
        wtn_all = [persist.tile([128, 512], F16, name=f"wtn{i}")
                   for i in range(8)]

        # ---------------- phase A ----------------
        # Q: host-pre-transposed fp16, packed (sg, t)-major so the first
        # DMA half covers everything attention qc=0 needs
        for sg in range(SQ // 512):
            for g in range(2):
                ps = gpsum.tile([128, 512], F32, tag="gp", name="ps")
                for t in range(2):
                    nc.tensor.matmul(
                        ps, wq_hf[t][:, g * 128:(g + 1) * 128],
                        qx_sb[:, (2 * sg + t) * 512:(2 * sg + t + 1) * 512],
                        start=(t == 0), stop=(t == 1))
                nc.vector.tensor_scalar_add(
                    qT_sb[g][:, sg * 512:(sg + 1) * 512], ps, bq_sb[:, g:g + 1])

        # K/V: host-compacted, host-transposed kvT [512 c, skc keys] fp16.
        # Direct 2D DMA per c-block; K blocks on sync, V on scalar queue.
        # kvT[p, j, n] = kv_row(idx[n])[j*128 + p]; j=0,1 are K, j=2,3 are V
        def emit_kv_chunk(cch):
            kvT = io_pool.tile([128, 4, 512], F16, tag="kvt", name="kvT",
                               bufs=2)
            nc.sync.dma_start(kvT[:, 0:2, :],
                              kvt_d[:, 0:2, 512 * cch:512 * (cch + 1)])
            nc.sync.dma_start(kvT[:, 2:4, :],
                              kvt_d[:, 2:4, 512 * cch:512 * (cch + 1)])
            for g in range(2):
                ps = gpsum.tile([128, 512], F32, tag="gp", name="ps")
                for t in range(2):
                    nc.tensor.matmul(
                        ps, wk_hf[t][:, g * 128:(g + 1) * 128], kvT[:, t, :],
                        start=(t == 0), stop=(t == 1))
                nc.vector.tensor_scalar_add(
                    kT_ch[g][cch], ps, bk_sb[:, g:g + 1])
            for j in range(4):
                s = cch * 4 + j
                vps = gpsum.tile([128, 512], F32, tag="gp", name="vps")[:, 0:256]
                for t in range(2):
                    nc.tensor.matmul(
                        vps, kvT[:, 2 + t, j * 128:(j + 1) * 128], wv_hf[t],
                        start=(t == 0), stop=(t == 1))
                dst = vaug[s].rearrange("p (h e) -> p h e", e=33)[:, :, 0:DH]
                src = vps.rearrange("p (h e) -> p h e", e=DH)
                nc.vector.tensor_copy(dst, src)

        # ---------------- attention ----------------
        def emit_attn_block(qc, g, wts, kt0, kt1, carry):
            """Scores + exp + PV for kt in [kt0, kt1), software-pipelined:
            scores(kt+1) are emitted before exp/PV(kt) so the PE never
            waits on the ACT exp chain. carry = [kt, sts] not yet PV'd."""
            for kt in range(kt0, kt1):
                sts = emit_scores(qc, g, kt)
                if carry[0] is not None:
                    emit_exp_pv(qc, g, wts, *carry)
                carry[0], carry[1] = kt, sts

        def emit_scores(qc, g, kt):
            sts = []
            for jj in range(2):
                st = st_pool.tile([128, 1024], F32, tag="st", name="st")
                for j2 in range(2):
                    j = 2 * jj + j2
                    nc.tensor.matmul(
                        st[:, j2 * 512:(j2 + 1) * 512],
                        kT_ch[g][kt // 4][32 * j:32 * j + 32,
                                          (kt % 4) * 128:
                                          (kt % 4) * 128 + 128],
                        qT_sb[g][32 * j:32 * j + 32,
                                 qc * 512:(qc + 1) * 512],
                        start=True, stop=True,
                        tile_position=(32 * j, 0))
                sts.append(st)
            return sts

        def emit_exp_pv(qc, g, wts, kt, sts):
            for jj in range(2):
                ptile = pt_pool.tile([128, 1024], F16, tag="pt",
                                     name="ptile")
                if kt < n_full:
                    nc.scalar.activation(ptile, sts[jj], AF.Exp,
                                         scale=SCALE)
                else:
                    nc.scalar.activation(ptile, sts[jj], AF.Exp,
                                         bias=maskbias[:, kt:kt + 1],
                                         scale=SCALE)
                for j2 in range(2):
                    h = 4 * g + 2 * jj + j2
                    nc.tensor.matmul(
                        wts[jj][64 * j2:64 * j2 + 33, :],
                        vaug[kt][:, 33 * h:33 * h + 33],
                        ptile[:, j2 * 512:(j2 + 1) * 512],
                        start=(kt == 0), stop=(kt == nkc - 1),
                        tile_position=(0, 64 * j2),
                        skip_group_check=True)

        def emit_attn_tail(qc, g, wts, carry):
            """Flush the pipelined last kt, then evict weightedT to SBUF
            wcop tiles on DVE; DMA the denominator rows straight out of
            the PSUM accumulators (strided 2-partition DMA)."""
            emit_exp_pv(qc, g, wts, *carry)
            wcops = []
            for jj in range(2):
                wcop = osb_pool.tile([128, 512], F32, tag="wcop", name="wcop")
                nc.vector.tensor_copy(wcop, wts[jj])
                nc.sync.dma_start(drows[qc][g][2 * jj:2 * jj + 2, :],
                                  wcop[32:97:64, :])
                wcops.append(wcop)
            wcop_all[qc][g] = wcops

        def emit_attn(qc, g):
            wts = [wt_pool.tile([128, 512], F32, tag="wt", name=f"wt{jj}")
                   for jj in range(2)]
            carry = [None, None]
            emit_attn_block(qc, g, wts, 0, nkc, carry)
            emit_attn_tail(qc, g, wts, carry)

        def emit_norm(qc, g):
            """Reciprocal + broadcast (fp32r matmul) + normalize for one
            (qc, head-group)."""
            rrec = rc_pool.tile([4, 512], F32, tag="rrec", name="rrec")
            nc.vector.reciprocal_approx_fast(out=rrec, in_=drows[qc][g])
            rrech = rc_pool.tile([4, 512], F16, tag="rrech", name="rrech")
            nc.vector.tensor_copy(rrech, rrec)
            for jj in range(2):
                u = 2 * g + jj
                bc = gpsum.tile([128, 512], F32, tag="gp", name="bc")
                nc.tensor.matmul(bc, sel4[jj], rrech, start=True, stop=True)
                wtn = wtn_all[4 * qc + u]
                nc.vector.tensor_mul(wtn, wcop_all[qc][g][jj], bc)

        def emit_outproj(qc):
            wtns = [wtn_all[4 * qc + u] for u in range(4)]
            for m in range(4):
                ops = gpsum.tile([128, 512], F32, tag="gp", name="ops")[:, 0:256]
                for p in range(4):
                    nc.tensor.matmul(
                        ops, wtns[p][:, m * 128:(m + 1) * 128],
                        wo_hf[:, p * 256:(p + 1) * 256],
                        start=(p == 0), stop=(p == 3),
                        skip_group_check=True)
                ob = ob_pool.tile([128, 256], F32, tag="ob", name="ob")
                nc.vector.tensor_add(ob, ops, bo_sb)
                (nc.scalar if m % 2 else nc.sync).dma_start(
                    out_d[qc * 512 + m * 128:qc * 512 + (m + 1) * 128, :],
                    ob)

        drows = [[rc_pool.tile([4, 512], F32, tag=f"drows{qc}_{g}",
                               name=f"drows{qc}_{g}") for g in range(2)]
                 for qc in range(2)]
        wcop_all = [[None, None], [None, None]]

        # attention (0,0) interleaved with phase-A K/V chunks so the PE
        # fills gather waits with ready score work
        wts00 = [wt_pool.tile([128, 512], F32, tag="wt", name=f"wt00_{jj}")
                 for jj in range(2)]
        carry00 = [None, None]
        for cch in range(skc // 512):
            emit_kv_chunk(cch)
            emit_attn_block(0, 0, wts00, cch * 4, cch * 4 + 4, carry00)
        emit_attn_tail(0, 0, wts00, carry00)

        emit_attn(0, 1)
        emit_norm(0, 0)
        emit_attn(1, 0)
        emit_norm(0, 1)
        emit_outproj(0)
        emit_attn(1, 1)
        emit_norm(1, 0)
        emit_norm(1, 1)
        emit_outproj(1)

    nc.finalize()
    return nc


def _get_nc(nkc, n_full):
    key = ("nc", nkc, n_full)
    if key not in _CACHE:
        _CACHE[key] = _build_nc(nkc, n_full)
    return _CACHE[key]


def kernel(query, key, value, mask, Wq, bq, Wk, bk, Wv, bv, Wo, bo,
           _trace=False):
    query = np.asarray(query, np.float32)
    key = np.asarray(key, np.float32)
    value = np.asarray(value, np.float32)
    mask = np.asarray(mask, np.int32)
    Wq = np.ascontiguousarray(np.asarray(Wq, np.float16))
    Wk = np.ascontiguousarray(np.asarray(Wk, np.float16))
    Wv = np.ascontiguousarray(np.asarray(Wv, np.float16))
    Wo32 = np.asarray(Wo, np.float32)
    bq = np.asarray(bq, np.float32)
    bk = np.asarray(bk, np.float32)
    bv = np.asarray(bv, np.float32)
    bo = np.asarray(bo, np.float32)

    # mask compaction: indices of surviving keys per batch, padded to a
    # multiple of 512 with a duplicate (masked-out) index
    idxs = [np.nonzero(mask[b, 0])[0].astype(np.int32) for b in range(2)]
    nk_max = max(len(ix) for ix in idxs)
    nk_max = max(nk_max, 1)
    skc = ((nk_max + 511) // 512) * 512
    nkc = skc // 128
    # tiles [0, n_full) contain no padded keys on ANY core (bias-free exp)
    n_full = min(len(ix) for ix in idxs) // 128

    nc = _get_nc(nkc, n_full)

    wo_arr = np.zeros((128, 4, 256), np.float32)
    for p in range(4):
        wo_arr[0:32, p] = Wo32[64 * p:64 * p + 32]
        wo_arr[64:96, p] = Wo32[64 * p + 32:64 * p + 64]
    wo_arr = wo_arr.reshape(128, 1024).astype(np.float16)

    # packed fp16 consts: wq | wk | wv (rows t-major) then wo_arr
    ch = np.zeros((128, 2816), np.float16)
    for t in range(2):
        ch[:, 256 * t:256 * (t + 1)] = Wq[128 * t:128 * (t + 1)]
        ch[:, 512 + 256 * t:512 + 256 * (t + 1)] = Wk[128 * t:128 * (t + 1)]
        ch[:, 1024 + 256 * t:1024 + 256 * (t + 1)] = Wv[128 * t:128 * (t + 1)]
    ch[:, 1536:2560] = wo_arr
    sel_f = np.zeros((128, 2, 128), np.float16)
    for u in range(2):
        sel_f[2 * u, u, 0:DH] = 1.0
        sel_f[2 * u + 1, u, 64:64 + DH] = 1.0
    ch[:, 2560:2816] = sel_f.reshape(128, 256)
    ch = np.ascontiguousarray(ch)

    # packed fp32 consts per batch: bq2 | bk2 | bo_rep | maskbias | sel
    bq2 = bq.reshape(2, 128).T
    bk2 = bk.reshape(2, 128).T
    bo_rep = np.broadcast_to((bv @ Wo32 + bo).reshape(1, 256), (128, 256))

    kv = [np.concatenate([key[b], value[b]], axis=1).astype(np.float16)
          for b in range(2)]
    qt2 = []
    for b in range(2):
        row = []
        for qi in range(4):
            qT = query[b, qi * SQ:(qi + 1) * SQ].T.astype(np.float16)
            row.append(np.ascontiguousarray(np.concatenate(
                [qT[128 * t:128 * (t + 1), 512 * sg:512 * (sg + 1)]
                 for sg in range(2) for t in range(2)], axis=1)))
        qt2.append(row)

    # host-side compaction + transpose: kvt5 [128, 4, skc] per batch
    kvt = []
    cfs = []
    for b in range(2):
        ix = idxs[b]
        nk = len(ix)
        ix_pad = np.concatenate(
            [ix, np.full(skc - nk, ix[0] if nk else 0, np.int32)])
        kvT = kv[b][ix_pad].T  # [512, skc]
        kvt.append(np.ascontiguousarray(kvT.reshape(4, 128, skc)
                                        .transpose(1, 0, 2)))
        mcomp = np.where(np.arange(skc) < nk, 0.0, NEG).astype(np.float32)
        mb = mcomp.reshape(nkc, 128).T
        cf = np.zeros((128, 516 + nkc), np.float32)
        cf[:, 0:2] = bq2
        cf[:, 2:4] = bk2
        cf[:, 4:260] = bo_rep
        cf[:, 260:260 + nkc] = mb
        cfs.append(np.ascontiguousarray(cf))

    in_maps = []
    for c in range(8):
        b, qi = divmod(c, 4)
        in_maps.append({
            "qt_in": qt2[b][qi],
            "kvt_in": kvt[b],
            "ch_in": ch,
            "cf_in": cfs[b],
        })

    res = run_bass_kernel_spmd(nc, in_maps, core_ids=list(range(8)),
                               trace=_trace)
    if _trace:
        _CACHE["last_result"] = res

    out = np.empty((2, 4096, HID), np.float32)
    for c in range(8):
        b, qi = divmod(c, 4)
        out[b, qi * SQ:(qi + 1) * SQ] = res.results[c]["out"]
    return out


# revision 38
# speedup vs baseline: 1.1387x; 1.0092x over previous
"""Trainium2 Bass kernel for nn_AttentionBlock (B=2, S=4096, HID=256, 8 heads).

Sharding: 8 cores = 2 batches x 4 query-chunks of 1024 queries.
Each core redundantly computes full K/V projections for its batch, then
attention for its 1024 queries over all 8 heads, then the output projection.
Host gathers by concatenation (no cross-core reduction needed).

Mask compaction: the (b,1,S) key mask zeroes whole keys for every query and
head, so masked keys are dropped up front. The host computes the surviving
key indices (padded to a multiple of 512; padding slots carry maskbias -1e4
so they contribute exactly zero). The device pulls the surviving K/V rows
with dma_gather(transpose=True) over a host-merged fp16 [S, 512] K||V
tensor: one SWDGE op per 512 keys that gathers AND transposes, landing
kvT [c, keys] tiles directly (no PE transposes, no evict copies).

Host-side prep: qT pre-transposed fp16 [256, 1024]; weights cast fp16;
maskbias {0,-1e4} fp32 precomputed; out-proj bias replicated [128,256];
gather indices int16 in the 16-partition wrapped layout.

Device pipeline per core (fp16 matmul datapath, fp32 PSUM accumulation):
  - Scores transposed: sT[k, q] via 4-way row-strip K=32 fp16 matmuls.
  - exp on ACT with per-partition mask bias and 1/sqrt(32) folded in.
  - PV fp16 matmuls col-packed in pairs with a ones-augmented V column per
    head, so the softmax denominator accumulates alongside weightedT.
  - Normalization: weightedT evicted to SBUF (ACT), denominator rows
    DMA-packed into one [8,512] tile, one reciprocal_approx_fast, K=8
    selector matmuls broadcast both head reciprocals per tile, one
    full-tile DVE multiply. Emission deferred past the next qc's first
    head group so the PE never stalls on it.
  - Output projection from stacked weightedT against zero-padded Wo;
    bias added on DVE during PSUM eviction.
"""

import numpy as np

import concourse.bacc as bacc
import concourse.bass as bass
from concourse import mybir
from concourse.tile import TileContext
from concourse.bass_utils import run_bass_kernel_spmd

F32 = mybir.dt.float32
F32R = mybir.dt.float32r
F16 = mybir.dt.float16
I16 = mybir.dt.int16
AF = mybir.ActivationFunctionType

HID = 256
HEADS = 8
DH = 32
SK = 4096
SQ = 1024  # queries per core
SCALE = 1.0 / np.sqrt(32.0)
NEG = -10000.0

_CACHE = {}


def _build_nc(nkc, n_full):
    """nkc = number of 128-key tiles after mask compaction (multiple of 4).
    n_full = number of those tiles with no padded keys (maskbias all zero)."""
    skc = nkc * 128
    nc = bacc.Bacc("TRN2", target_bir_lowering=False, debug=False, num_devices=8)

    qt_d = nc.dram_tensor("qt_in", [128, 2 * SQ], F16, kind="ExternalInput").ap()
    kvt_d = nc.dram_tensor("kvt_in", [128, 4, skc], F16, kind="ExternalInput").ap()
    # packed fp16 consts: wq|wk|wv (6 x 256) then wo_arr (1024) then sel (256)
    ch_d = nc.dram_tensor("ch_in", [128, 2816], F16, kind="ExternalInput").ap()
    # packed fp32 consts: bq2 (2) | bk2 (2) | bo_rep (256) | maskbias (nkc)
    # | sel (2 x 128, rows 0-3 only)
    cf_d = nc.dram_tensor("cf_in", [128, 516 + nkc], F32, kind="ExternalInput").ap()
    out_d = nc.dram_tensor("out", [SQ, HID], F32, kind="ExternalOutput").ap()

    from contextlib import ExitStack

    with TileContext(nc) as tc, ExitStack() as top:
        const = top.enter_context(tc.tile_pool(name="const", bufs=1))
        persist = top.enter_context(tc.tile_pool(name="persist", bufs=1))
        io_pool = top.enter_context(tc.tile_pool(name="io", bufs=3))
        pt_pool = top.enter_context(tc.tile_pool(name="pt", bufs=4))
        rc_pool = top.enter_context(tc.tile_pool(name="rc", bufs=2))
        osb_pool = top.enter_context(tc.tile_pool(name="osb", bufs=8))
        ob_pool = top.enter_context(tc.tile_pool(name="ob", bufs=4))

        gpsum = top.enter_context(tc.tile_pool(name="gpsum", bufs=2, space="PSUM"))
        st_pool = top.enter_context(tc.tile_pool(name="stp", bufs=2, space="PSUM"))
        wt_pool = top.enter_context(tc.tile_pool(name="wtp", bufs=2, space="PSUM"))

        # ---------------- constants ----------------
        # three packed const DMAs on the Activation hwdge queue (the sync
        # queue carries the kvT chunks that gate attention start)
        qx_sb = const.tile([128, 2 * SQ], F16, name="qx_sb")
        nc.scalar.dma_start(qx_sb[:, 0:SQ], qt_d[:, 0:SQ])
        ch_sb = const.tile([128, 2816], F16, name="ch_sb")
        nc.scalar.dma_start(ch_sb[:, 0:1024], ch_d[:, 0:1024])
        cf_sb = const.tile([128, 516 + nkc], F32, name="cf_sb")
        nc.scalar.dma_start(cf_sb, cf_d)
        nc.scalar.dma_start(ch_sb[:, 1024:1536], ch_d[:, 1024:1536])
        nc.scalar.dma_start(qx_sb[:, SQ:2 * SQ], qt_d[:, SQ:2 * SQ])
        nc.scalar.dma_start(ch_sb[:, 1536:2816], ch_d[:, 1536:2816])

        wq_hf = [ch_sb[:, 0:256], ch_sb[:, 256:512]]
        wk_hf = [ch_sb[:, 512:768], ch_sb[:, 768:1024]]
        wv_hf = [ch_sb[:, 1024:1280], ch_sb[:, 1280:1536]]
        wo_hf = ch_sb[:, 1536:2560]
        sel2 = [ch_sb[0:2, 2560:2688]] * 2
        bq_sb = cf_sb[:, 0:2]
        bk_sb = cf_sb[:, 2:4]
        bo_sb = cf_sb[:, 4:260]
        maskbias = cf_sb[:, 260:260 + nkc]

        # ---------------- persistent buffers ----------------
        qT_sb = [persist.tile([128, SQ], F16, name=f"qT_sb{g}") for g in range(2)]
        kT_ch = [[persist.tile([128, 512], F16, name=f"kT{g}_{c}")
                  for c in range(skc // 512)] for g in range(2)]
        # augmented V, one [128, 264] tile per ktile (ones in column 33h+32)
        vaug = [persist.tile([128, 264], F16, name=f"vaug{s}")
                for s in range(nkc)]
        for s in range(nkc):
            nc.gpsimd.memset(vaug[s], 1.0)
        wtn_all = [persist.tile([128, 512], F16, name=f"wtn{i}")
                   for i in range(8)]

        # ---------------- phase A ----------------
        # Q: host-pre-transposed fp16, packed (sg, t)-major so the first
        # DMA half covers everything attention qc=0 needs
        for sg in range(SQ // 512):
            for g in range(2):
                ps = gpsum.tile([128, 512], F32, tag="gp", name="ps")
                for t in range(2):
                    nc.tensor.matmul(
                        ps, wq_hf[t][:, g * 128:(g + 1) * 128],
                        qx_sb[:, (2 * sg + t) * 512:(2 * sg + t + 1) * 512],
                        start=(t == 0), stop=(t == 1))
                nc.vector.tensor_scalar_add(
                    qT_sb[g][:, sg * 512:(sg + 1) * 512], ps, bq_sb[:, g:g + 1])

        # K/V: host-compacted, host-transposed kvT [512 c, skc keys] fp16.
        # Direct 2D DMA per c-block; K blocks on sync, V on scalar queue.
        # kvT[p, j, n] = kv_row(idx[n])[j*128 + p]; j=0,1 are K, j=2,3 are V
        def emit_kv_chunk(cch):
            kvT = io_pool.tile([128, 4, 512], F16, tag="kvt", name="kvT",
                               bufs=2)
            nc.sync.dma_start(kvT[:, 0:2, :],
                              kvt_d[:, 0:2, 512 * cch:512 * (cch + 1)])
            nc.sync.dma_start(kvT[:, 2:4, :],
                              kvt_d[:, 2:4, 512 * cch:512 * (cch + 1)])
            for g in range(2):
                ps = gpsum.tile([128, 512], F32, tag="gp", name="ps")
                for t in range(2):
                    nc.tensor.matmul(
                        ps, wk_hf[t][:, g * 128:(g + 1) * 128], kvT[:, t, :],
                        start=(t == 0), stop=(t == 1))
                nc.vector.tensor_scalar_add(
                    kT_ch[g][cch], ps, bk_sb[:, g:g + 1])
            for j in range(4):
                s = cch * 4 + j
                vps = gpsum.tile([128, 512], F32, tag="gp", name="vps")[:, 0:256]
                for t in range(2):
                    nc.tensor.matmul(
                        vps, kvT[:, 2 + t, j * 128:(j + 1) * 128], wv_hf[t],
                        start=(t == 0), stop=(t == 1))
                dst = vaug[s].rearrange("p (h e) -> p h e", e=33)[:, :, 0:DH]
                src = vps.rearrange("p (h e) -> p h e", e=DH)
                nc.vector.tensor_copy(dst, src)

        # ---------------- attention ----------------
        def emit_attn_block(qc, g, wts, kt0, kt1, carry):
            """Scores + exp + PV for kt in [kt0, kt1), software-pipelined:
            scores(kt+1) are emitted before exp/PV(kt) so the PE never
            waits on the ACT exp chain. carry = [kt, sts] not yet PV'd."""
            for kt in range(kt0, kt1):
                sts = emit_scores(qc, g, kt)
                if carry[0] is not None:
                    emit_exp_pv(qc, g, wts, *carry)
                carry[0], carry[1] = kt, sts

        def emit_scores(qc, g, kt):
            sts = []
            for jj in range(2):
                st = st_pool.tile([128, 1024], F32, tag="st", name="st")
                for j2 in range(2):
                    j = 2 * jj + j2
                    nc.tensor.matmul(
                        st[:, j2 * 512:(j2 + 1) * 512],
                        kT_ch[g][kt // 4][32 * j:32 * j + 32,
                                          (kt % 4) * 128:
                                          (kt % 4) * 128 + 128],
                        qT_sb[g][32 * j:32 * j + 32,
                                 qc * 512:(qc + 1) * 512],
                        start=True, stop=True,
                        tile_position=(32 * j, 0))
                sts.append(st)
            return sts

        def emit_exp_pv(qc, g, wts, kt, sts):
            for jj in range(2):
                ptile = pt_pool.tile([128, 1024], F16, tag="pt",
                                     name="ptile")
                if kt < n_full:
                    nc.scalar.activation(ptile, sts[jj], AF.Exp,
                                         scale=SCALE)
                else:
                    nc.scalar.activation(ptile, sts[jj], AF.Exp,
                                         bias=maskbias[:, kt:kt + 1],
                                         scale=SCALE)
                for j2 in range(2):
                    h = 4 * g + 2 * jj + j2
                    nc.tensor.matmul(
                        wts[jj][64 * j2:64 * j2 + 33, :],
                        vaug[kt][:, 33 * h:33 * h + 33],
                        ptile[:, j2 * 512:(j2 + 1) * 512],
                        start=(kt == 0), stop=(kt == nkc - 1),
                        tile_position=(0, 64 * j2),
                        skip_group_check=True)

        def emit_attn_tail(qc, g, wts, carry):
            """Flush the pipelined last kt, then evict weightedT to SBUF
            wcop tiles on DVE; DMA the denominator rows straight out of
            the PSUM accumulators (strided 2-partition DMA)."""
            emit_exp_pv(qc, g, wts, *carry)
            wcops = []
            for jj in range(2):
                wcop = osb_pool.tile([128, 512], F32, tag="wcop", name="wcop")
                nc.vector.tensor_copy(wcop, wts[jj])
                nc.sync.dma_start(drows[qc][g][jj],
                                  wcop[32:97:64, :])
                wcops.append(wcop)
            wcop_all[qc][g] = wcops

        def emit_attn(qc, g):
            wts = [wt_pool.tile([128, 512], F32, tag="wt", name=f"wt{jj}")
                   for jj in range(2)]
            carry = [None, None]
            emit_attn_block(qc, g, wts, 0, nkc, carry)
            emit_attn_tail(qc, g, wts, carry)

        def emit_norm(qc, g):
            """Per-jj reciprocal + broadcast + normalize chains so each
            depends only on its own PV accumulator."""
            for jj in range(2):
                u = 2 * g + jj
                rrec = rc_pool.tile([2, 512], F32, tag="rrec", name="rrec")
                nc.vector.reciprocal_approx_fast(
                    out=rrec, in_=drows[qc][g][jj])
                rrech = rc_pool.tile([2, 512], F16, tag="rrech", name="rrech")
                nc.vector.tensor_copy(rrech, rrec)
                bc = gpsum.tile([128, 512], F32, tag="gp", name="bc")
                nc.tensor.matmul(bc, sel2[jj], rrech, start=True, stop=True)
                wtn = wtn_all[4 * qc + u]
                nc.vector.tensor_mul(wtn, wcop_all[qc][g][jj], bc)

        def emit_outproj(qc):
            wtns = [wtn_all[4 * qc + u] for u in range(4)]
            for m in range(4):
                ops = gpsum.tile([128, 512], F32, tag="gp", name="ops")[:, 0:256]
                for p in range(4):
                    nc.tensor.matmul(
                        ops, wtns[p][:, m * 128:(m + 1) * 128],
                        wo_hf[:, p * 256:(p + 1) * 256],
                        start=(p == 0), stop=(p == 3),
                        skip_group_check=True)
                ob = ob_pool.tile([128, 256], F32, tag="ob", name="ob")
                nc.vector.tensor_add(ob, ops, bo_sb)
                (nc.scalar if m % 2 else nc.sync).dma_start(
                    out_d[qc * 512 + m * 128:qc * 512 + (m + 1) * 128, :],
                    ob)

        drows = [[[rc_pool.tile([2, 512], F32, tag=f"drows{qc}_{g}_{jj}",
                                name=f"drows{qc}_{g}_{jj}")
                   for jj in range(2)] for g in range(2)] for qc in range(2)]
        wcop_all = [[None, None], [None, None]]

        # attention (0,0) interleaved with phase-A K/V chunks so the PE
        # fills gather waits with ready score work
        wts00 = [wt_pool.tile([128, 512], F32, tag="wt", name=f"wt00_{jj}")
                 for jj in range(2)]
        carry00 = [None, None]
        for cch in range(skc // 512):
            emit_kv_chunk(cch)
            emit_attn_block(0, 0, wts00, cch * 4, cch * 4 + 4, carry00)
        emit_attn_tail(0, 0, wts00, carry00)

        emit_attn(0, 1)
        emit_norm(0, 0)
        emit_attn(1, 0)
        emit_norm(0, 1)
        emit_outproj(0)
        emit_attn(1, 1)
        emit_norm(1, 0)
        emit_norm(1, 1)
        emit_outproj(1)

    nc.finalize()
    return nc


def _get_nc(nkc, n_full):
    key = ("nc", nkc, n_full)
    if key not in _CACHE:
        _CACHE[key] = _build_nc(nkc, n_full)
    return _CACHE[key]


def kernel(query, key, value, mask, Wq, bq, Wk, bk, Wv, bv, Wo, bo,
           _trace=False):
    query = np.asarray(query, np.float32)
    key = np.asarray(key, np.float32)
    value = np.asarray(value, np.float32)
    mask = np.asarray(mask, np.int32)
    Wq = np.ascontiguousarray(np.asarray(Wq, np.float16))
    Wk = np.ascontiguousarray(np.asarray(Wk, np.float16))
    Wv = np.ascontiguousarray(np.asarray(Wv, np.float16))
    Wo32 = np.asarray(Wo, np.float32)
    bq = np.asarray(bq, np.float32)
    bk = np.asarray(bk, np.float32)
    bv = np.asarray(bv, np.float32)
    bo = np.asarray(bo, np.float32)

    # mask compaction: indices of surviving keys per batch, padded to a
    # multiple of 512 with a duplicate (masked-out) index
    idxs = [np.nonzero(mask[b, 0])[0].astype(np.int32) for b in range(2)]
    nk_max = max(len(ix) for ix in idxs)
    nk_max = max(nk_max, 1)
    skc = ((nk_max + 511) // 512) * 512
    nkc = skc // 128
    # tiles [0, n_full) contain no padded keys on ANY core (bias-free exp)
    n_full = min(len(ix) for ix in idxs) // 128

    nc = _get_nc(nkc, n_full)

    wo_arr = np.zeros((128, 4, 256), np.float32)
    for p in range(4):
        wo_arr[0:32, p] = Wo32[64 * p:64 * p + 32]
        wo_arr[64:96, p] = Wo32[64 * p + 32:64 * p + 64]
    wo_arr = wo_arr.reshape(128, 1024).astype(np.float16)

    # packed fp16 consts: wq | wk | wv (rows t-major) then wo_arr
    ch = np.zeros((128, 2816), np.float16)
    for t in range(2):
        ch[:, 256 * t:256 * (t + 1)] = Wq[128 * t:128 * (t + 1)]
        ch[:, 512 + 256 * t:512 + 256 * (t + 1)] = Wk[128 * t:128 * (t + 1)]
        ch[:, 1024 + 256 * t:1024 + 256 * (t + 1)] = Wv[128 * t:128 * (t + 1)]
    ch[:, 1536:2560] = wo_arr
    sel_f = np.zeros((128, 256), np.float16)
    sel_f[0, 0:DH] = 1.0
    sel_f[1, 64:64 + DH] = 1.0
    ch[:, 2560:2816] = sel_f
    ch = np.ascontiguousarray(ch)

    # packed fp32 consts per batch: bq2 | bk2 | bo_rep | maskbias | sel
    bq2 = bq.reshape(2, 128).T
    bk2 = bk.reshape(2, 128).T
    bo_rep = np.broadcast_to((bv @ Wo32 + bo).reshape(1, 256), (128, 256))

    kv = [np.concatenate([key[b], value[b]], axis=1).astype(np.float16)
          for b in range(2)]
    qt2 = []
    for b in range(2):
        row = []
        for qi in range(4):
            qT = query[b, qi * SQ:(qi + 1) * SQ].T.astype(np.float16)
            row.append(np.ascontiguousarray(np.concatenate(
                [qT[128 * t:128 * (t + 1), 512 * sg:512 * (sg + 1)]
                 for sg in range(2) for t in range(2)], axis=1)))
        qt2.append(row)

    # host-side compaction + transpose: kvt5 [128, 4, skc] per batch
    kvt = []
    cfs = []
    for b in range(2):
        ix = idxs[b]
        nk = len(ix)
        ix_pad = np.concatenate(
            [ix, np.full(skc - nk, ix[0] if nk else 0, np.int32)])
        kvT = kv[b][ix_pad].T  # [512, skc]
        kvt.append(np.ascontiguousarray(kvT.reshape(4, 128, skc)
                                        .transpose(1, 0, 2)))
        mcomp = np.where(np.arange(skc) < nk, 0.0, NEG).astype(np.float32)
        mb = mcomp.reshape(nkc, 128).T
        cf = np.zeros((128, 516 + nkc), np.float32)
        cf[:, 0:2] = bq2
        cf[:, 2:4] = bk2
        cf[:, 4:260] = bo_rep
        cf[:, 260:260 + nkc] = mb
        cfs.append(np.ascontiguousarray(cf))

    in_maps = []
    for c in range(8):
        b, qi = divmod(c, 4)
        in_maps.append({
            "qt_in": qt2[b][qi],
            "kvt_in": kvt[b],
            "ch_in": ch,
            "cf_in": cfs[b],
        })

    res = run_bass_kernel_spmd(nc, in_maps, core_ids=list(range(8)),
                               trace=_trace)
    if _trace:
        _CACHE["last_result"] = res

    out = np.empty((2, 4096, HID), np.float32)
    for c in range(8):
        b, qi = divmod(c, 4)
        out[b, qi * SQ:(qi + 1) * SQ] = res.results[c]["out"]
    return out


# revision 39
# speedup vs baseline: 1.1581x; 1.0170x over previous
"""Trainium2 Bass kernel for nn_AttentionBlock (B=2, S=4096, HID=256, 8 heads).

Sharding: 8 cores = 2 batches x 4 query-chunks of 1024 queries.
Each core redundantly computes full K/V projections for its batch, then
attention for its 1024 queries over all 8 heads, then the output projection.
Host gathers by concatenation (no cross-core reduction needed).

Mask compaction: the (b,1,S) key mask zeroes whole keys for every query and
head, so masked keys are dropped up front. The host computes the surviving
key indices (padded to a multiple of 512; padding slots carry maskbias -1e4
so they contribute exactly zero). The device pulls the surviving K/V rows
with dma_gather(transpose=True) over a host-merged fp16 [S, 512] K||V
tensor: one SWDGE op per 512 keys that gathers AND transposes, landing
kvT [c, keys] tiles directly (no PE transposes, no evict copies).

Host-side prep: qT pre-transposed fp16 [256, 1024]; weights cast fp16;
maskbias {0,-1e4} fp32 precomputed; out-proj bias replicated [128,256];
gather indices int16 in the 16-partition wrapped layout.

Device pipeline per core (fp16 matmul datapath, fp32 PSUM accumulation):
  - Scores transposed: sT[k, q] via 4-way row-strip K=32 fp16 matmuls.
  - exp on ACT with per-partition mask bias and 1/sqrt(32) folded in.
  - PV fp16 matmuls col-packed in pairs with a ones-augmented V column per
    head, so the softmax denominator accumulates alongside weightedT.
  - Normalization: weightedT evicted to SBUF (ACT), denominator rows
    DMA-packed into one [8,512] tile, one reciprocal_approx_fast, K=8
    selector matmuls broadcast both head reciprocals per tile, one
    full-tile DVE multiply. Emission deferred past the next qc's first
    head group so the PE never stalls on it.
  - Output projection from stacked weightedT against zero-padded Wo;
    bias added on DVE during PSUM eviction.
"""

import numpy as np

import concourse.bacc as bacc
import concourse.bass as bass
from concourse import mybir
from concourse.tile import TileContext
from concourse.bass_utils import run_bass_kernel_spmd

F32 = mybir.dt.float32
F32R = mybir.dt.float32r
F16 = mybir.dt.float16
I16 = mybir.dt.int16
AF = mybir.ActivationFunctionType

HID = 256
HEADS = 8
DH = 32
SK = 4096
SQ = 1024  # queries per core
SCALE = 1.0 / np.sqrt(32.0)
NEG = -10000.0

_CACHE = {}


def _build_nc(nkc, n_full):
    """nkc = number of 128-key tiles after mask compaction (multiple of 4).
    n_full = number of those tiles with no padded keys (maskbias all zero)."""
    skc = nkc * 128
    nc = bacc.Bacc("TRN2", target_bir_lowering=False, debug=False, num_devices=8)

    qt_d = nc.dram_tensor("qt_in", [128, 2 * SQ], F16, kind="ExternalInput").ap()
    kvt_d = nc.dram_tensor("kvt_in", [128, 4, skc], F16, kind="ExternalInput").ap()
    # packed fp16 consts: wq|wk|wv (6 x 256) then wo_arr (1024) then sel (256)
    ch_d = nc.dram_tensor("ch_in", [128, 2816], F16, kind="ExternalInput").ap()
    # packed fp32 consts: bq2 (2) | bk2 (2) | bo_rep (256) | maskbias (nkc)
    # | sel (2 x 128, rows 0-3 only)
    cf_d = nc.dram_tensor("cf_in", [128, 516 + nkc], F32, kind="ExternalInput").ap()
    out_d = nc.dram_tensor("out", [SQ, HID], F32, kind="ExternalOutput").ap()

    from contextlib import ExitStack

    with TileContext(nc) as tc, ExitStack() as top:
        const = top.enter_context(tc.tile_pool(name="const", bufs=1))
        persist = top.enter_context(tc.tile_pool(name="persist", bufs=1))
        io_pool = top.enter_context(tc.tile_pool(name="io", bufs=3))
        pt_pool = top.enter_context(tc.tile_pool(name="pt", bufs=4))
        rc_pool = top.enter_context(tc.tile_pool(name="rc", bufs=2))
        osb_pool = top.enter_context(tc.tile_pool(name="osb", bufs=8))
        ob_pool = top.enter_context(tc.tile_pool(name="ob", bufs=4))

        gpsum = top.enter_context(tc.tile_pool(name="gpsum", bufs=2, space="PSUM"))
        st_pool = top.enter_context(tc.tile_pool(name="stp", bufs=2, space="PSUM"))
        wt_pool = top.enter_context(tc.tile_pool(name="wtp", bufs=2, space="PSUM"))

        # ---------------- constants ----------------
        # three packed const DMAs on the Activation hwdge queue (the sync
        # queue carries the kvT chunks that gate attention start)
        qx_sb = const.tile([128, 2 * SQ], F16, name="qx_sb")
        nc.scalar.dma_start(qx_sb[:, 0:SQ], qt_d[:, 0:SQ])
        ch_sb = const.tile([128, 2816], F16, name="ch_sb")
        nc.scalar.dma_start(ch_sb[:, 0:1024], ch_d[:, 0:1024])
        cf_sb = const.tile([128, 516 + nkc], F32, name="cf_sb")
        nc.scalar.dma_start(cf_sb, cf_d)
        nc.scalar.dma_start(ch_sb[:, 1024:1536], ch_d[:, 1024:1536])
        nc.scalar.dma_start(qx_sb[:, SQ:2 * SQ], qt_d[:, SQ:2 * SQ])
        nc.scalar.dma_start(ch_sb[:, 1536:2816], ch_d[:, 1536:2816])

        wq_hf = [ch_sb[:, 0:256], ch_sb[:, 256:512]]
        wk_hf = [ch_sb[:, 512:768], ch_sb[:, 768:1024]]
        wv_hf = [ch_sb[:, 1024:1280], ch_sb[:, 1280:1536]]
        wo_hf = ch_sb[:, 1536:2560]
        sel2 = [ch_sb[0:2, 2560:2688]] * 2
        bq_sb = cf_sb[:, 0:2]
        bk_sb = cf_sb[:, 2:4]
        bo_sb = cf_sb[:, 4:260]
        maskbias = cf_sb[:, 260:260 + nkc]

        # ---------------- persistent buffers ----------------
        qT_sb = [persist.tile([128, SQ], F16, name=f"qT_sb{g}") for g in range(2)]
        kT_ch = [[persist.tile([128, 512], F16, name=f"kT{g}_{c}")
                  for c in range(skc // 512)] for g in range(2)]
        # augmented V, one [128, 264] tile per ktile (ones in column 33h+32)
        vaug = [persist.tile([128, 264], F16, name=f"vaug{s}")
                for s in range(nkc)]
        for s in range(nkc):
            nc.gpsimd.memset(vaug[s], 1.0)
        wtn_all = [persist.tile([128, 512], F16, name=f"wtn{i}")
                   for i in range(8)]

        # ---------------- phase A ----------------
        # Q: host-pre-transposed fp16, packed (sg, t)-major so the first
        # DMA half covers everything attention qc=0 needs. sg=1 is emitted
        # mid-chunk-loop (its qx half arrives late by design).
        def emit_qproj(sg):
            for g in range(2):
                ps = gpsum.tile([128, 512], F32, tag="gp", name="ps")
                for t in range(2):
                    nc.tensor.matmul(
                        ps, wq_hf[t][:, g * 128:(g + 1) * 128],
                        qx_sb[:, (2 * sg + t) * 512:(2 * sg + t + 1) * 512],
                        start=(t == 0), stop=(t == 1))
                nc.vector.tensor_scalar_add(
                    qT_sb[g][:, sg * 512:(sg + 1) * 512], ps, bq_sb[:, g:g + 1])

        emit_qproj(0)

        # K/V: host-compacted, host-transposed kvT [512 c, skc keys] fp16.
        # Direct 2D DMA per c-block; K blocks on sync, V on scalar queue.
        # kvT[p, j, n] = kv_row(idx[n])[j*128 + p]; j=0,1 are K, j=2,3 are V
        def emit_kv_chunk(cch):
            kvT = io_pool.tile([128, 4, 512], F16, tag="kvt", name="kvT",
                               bufs=2)
            nc.sync.dma_start(kvT[:, 0:2, :],
                              kvt_d[:, 0:2, 512 * cch:512 * (cch + 1)])
            nc.sync.dma_start(kvT[:, 2:4, :],
                              kvt_d[:, 2:4, 512 * cch:512 * (cch + 1)])
            for g in range(2):
                ps = gpsum.tile([128, 512], F32, tag="gp", name="ps")
                for t in range(2):
                    nc.tensor.matmul(
                        ps, wk_hf[t][:, g * 128:(g + 1) * 128], kvT[:, t, :],
                        start=(t == 0), stop=(t == 1))
                nc.vector.tensor_scalar_add(
                    kT_ch[g][cch], ps, bk_sb[:, g:g + 1])
            for j in range(4):
                s = cch * 4 + j
                vps = gpsum.tile([128, 512], F32, tag="gp", name="vps")[:, 0:256]
                for t in range(2):
                    nc.tensor.matmul(
                        vps, kvT[:, 2 + t, j * 128:(j + 1) * 128], wv_hf[t],
                        start=(t == 0), stop=(t == 1))
                dst = vaug[s].rearrange("p (h e) -> p h e", e=33)[:, :, 0:DH]
                src = vps.rearrange("p (h e) -> p h e", e=DH)
                nc.vector.tensor_copy(dst, src)

        # ---------------- attention ----------------
        def emit_attn_block(qc, g, wts, kt0, kt1, carry):
            """Scores + exp + PV for kt in [kt0, kt1), software-pipelined:
            scores(kt+1) are emitted before exp/PV(kt) so the PE never
            waits on the ACT exp chain. carry = [kt, sts] not yet PV'd."""
            for kt in range(kt0, kt1):
                sts = emit_scores(qc, g, kt)
                if carry[0] is not None:
                    emit_exp_pv(qc, g, wts, *carry)
                carry[0], carry[1] = kt, sts

        def emit_scores(qc, g, kt):
            sts = []
            for jj in range(2):
                st = st_pool.tile([128, 1024], F32, tag="st", name="st")
                for j2 in range(2):
                    j = 2 * jj + j2
                    nc.tensor.matmul(
                        st[:, j2 * 512:(j2 + 1) * 512],
                        kT_ch[g][kt // 4][32 * j:32 * j + 32,
                                          (kt % 4) * 128:
                                          (kt % 4) * 128 + 128],
                        qT_sb[g][32 * j:32 * j + 32,
                                 qc * 512:(qc + 1) * 512],
                        start=True, stop=True,
                        tile_position=(32 * j, 0))
                sts.append(st)
            return sts

        def emit_exp_pv(qc, g, wts, kt, sts):
            for jj in range(2):
                ptile = pt_pool.tile([128, 1024], F16, tag="pt",
                                     name="ptile")
                if kt < n_full:
                    nc.scalar.activation(ptile, sts[jj], AF.Exp,
                                         scale=SCALE)
                else:
                    nc.scalar.activation(ptile, sts[jj], AF.Exp,
                                         bias=maskbias[:, kt:kt + 1],
                                         scale=SCALE)
                for j2 in range(2):
                    h = 4 * g + 2 * jj + j2
                    nc.tensor.matmul(
                        wts[jj][64 * j2:64 * j2 + 33, :],
                        vaug[kt][:, 33 * h:33 * h + 33],
                        ptile[:, j2 * 512:(j2 + 1) * 512],
                        start=(kt == 0), stop=(kt == nkc - 1),
                        tile_position=(0, 64 * j2),
                        skip_group_check=True)

        def emit_attn_tail(qc, g, wts, carry):
            """Flush the pipelined last kt, then evict weightedT to SBUF
            wcop tiles on DVE; DMA the denominator rows straight out of
            the PSUM accumulators (strided 2-partition DMA)."""
            emit_exp_pv(qc, g, wts, *carry)
            wcops = []
            for jj in range(2):
                wcop = osb_pool.tile([128, 512], F32, tag="wcop", name="wcop")
                nc.vector.tensor_copy(wcop, wts[jj])
                nc.sync.dma_start(drows[qc][g][jj],
                                  wcop[32:97:64, :])
                wcops.append(wcop)
            wcop_all[qc][g] = wcops

        def emit_attn(qc, g):
            wts = [wt_pool.tile([128, 512], F32, tag="wt", name=f"wt{jj}")
                   for jj in range(2)]
            carry = [None, None]
            emit_attn_block(qc, g, wts, 0, nkc, carry)
            emit_attn_tail(qc, g, wts, carry)

        def emit_norm(qc, g):
            """Per-jj reciprocal + broadcast + normalize chains so each
            depends only on its own PV accumulator."""
            for jj in range(2):
                u = 2 * g + jj
                rrec = rc_pool.tile([2, 512], F32, tag="rrec", name="rrec")
                nc.vector.reciprocal_approx_fast(
                    out=rrec, in_=drows[qc][g][jj])
                rrech = rc_pool.tile([2, 512], F16, tag="rrech", name="rrech")
                nc.vector.tensor_copy(rrech, rrec)
                bc = gpsum.tile([128, 512], F32, tag="gp", name="bc")
                nc.tensor.matmul(bc, sel2[jj], rrech, start=True, stop=True)
                wtn = wtn_all[4 * qc + u]
                nc.vector.tensor_mul(wtn, wcop_all[qc][g][jj], bc)

        def emit_outproj(qc):
            wtns = [wtn_all[4 * qc + u] for u in range(4)]
            for m in range(4):
                ops = gpsum.tile([128, 512], F32, tag="gp", name="ops")[:, 0:256]
                for p in range(4):
                    nc.tensor.matmul(
                        ops, wtns[p][:, m * 128:(m + 1) * 128],
                        wo_hf[:, p * 256:(p + 1) * 256],
                        start=(p == 0), stop=(p == 3),
                        skip_group_check=True)
                ob = ob_pool.tile([128, 256], F32, tag="ob", name="ob")
                nc.vector.tensor_add(ob, ops, bo_sb)
                (nc.scalar if m % 2 else nc.sync).dma_start(
                    out_d[qc * 512 + m * 128:qc * 512 + (m + 1) * 128, :],
                    ob)

        drows = [[[rc_pool.tile([2, 512], F32, tag=f"drows{qc}_{g}_{jj}",
                                name=f"drows{qc}_{g}_{jj}")
                   for jj in range(2)] for g in range(2)] for qc in range(2)]
        wcop_all = [[None, None], [None, None]]

        # attention (0,0) interleaved with phase-A K/V chunks so the PE
        # fills gather waits with ready score work
        wts00 = [wt_pool.tile([128, 512], F32, tag="wt", name=f"wt00_{jj}")
                 for jj in range(2)]
        carry00 = [None, None]
        for cch in range(skc // 512):
            emit_kv_chunk(cch)
            emit_attn_block(0, 0, wts00, cch * 4, cch * 4 + 4, carry00)
            if cch == 0:
                emit_qproj(1)
        emit_attn_tail(0, 0, wts00, carry00)

        emit_attn(0, 1)
        emit_norm(0, 0)
        emit_attn(1, 0)
        emit_norm(0, 1)
        emit_outproj(0)
        emit_attn(1, 1)
        emit_norm(1, 0)
        emit_norm(1, 1)
        emit_outproj(1)

    nc.finalize()
    return nc


def _get_nc(nkc, n_full):
    key = ("nc", nkc, n_full)
    if key not in _CACHE:
        _CACHE[key] = _build_nc(nkc, n_full)
    return _CACHE[key]


def kernel(query, key, value, mask, Wq, bq, Wk, bk, Wv, bv, Wo, bo,
           _trace=False):
    query = np.asarray(query, np.float32)
    key = np.asarray(key, np.float32)
    value = np.asarray(value, np.float32)
    mask = np.asarray(mask, np.int32)
    Wq = np.ascontiguousarray(np.asarray(Wq, np.float16))
    Wk = np.ascontiguousarray(np.asarray(Wk, np.float16))
    Wv = np.ascontiguousarray(np.asarray(Wv, np.float16))
    Wo32 = np.asarray(Wo, np.float32)
    bq = np.asarray(bq, np.float32)
    bk = np.asarray(bk, np.float32)
    bv = np.asarray(bv, np.float32)
    bo = np.asarray(bo, np.float32)

    # mask compaction: indices of surviving keys per batch, padded to a
    # multiple of 512 with a duplicate (masked-out) index
    idxs = [np.nonzero(mask[b, 0])[0].astype(np.int32) for b in range(2)]
    nk_max = max(len(ix) for ix in idxs)
    nk_max = max(nk_max, 1)
    skc = ((nk_max + 511) // 512) * 512
    nkc = skc // 128
    # tiles [0, n_full) contain no padded keys on ANY core (bias-free exp)
    n_full = min(len(ix) for ix in idxs) // 128

    nc = _get_nc(nkc, n_full)

    wo_arr = np.zeros((128, 4, 256), np.float32)
    for p in range(4):
        wo_arr[0:32, p] = Wo32[64 * p:64 * p + 32]
        wo_arr[64:96, p] = Wo32[64 * p + 32:64 * p + 64]
    wo_arr = wo_arr.reshape(128, 1024).astype(np.float16)

    # packed fp16 consts: wq | wk | wv (rows t-major) then wo_arr
    ch = np.zeros((128, 2816), np.float16)
    for t in range(2):
        ch[:, 256 * t:256 * (t + 1)] = Wq[128 * t:128 * (t + 1)]
        ch[:, 512 + 256 * t:512 + 256 * (t + 1)] = Wk[128 * t:128 * (t + 1)]
        ch[:, 1024 + 256 * t:1024 + 256 * (t + 1)] = Wv[128 * t:128 * (t + 1)]
    ch[:, 1536:2560] = wo_arr
    sel_f = np.zeros((128, 256), np.float16)
    sel_f[0, 0:DH] = 1.0
    sel_f[1, 64:64 + DH] = 1.0
    ch[:, 2560:2816] = sel_f
    ch = np.ascontiguousarray(ch)

    # packed fp32 consts per batch: bq2 | bk2 | bo_rep | maskbias | sel
    bq2 = bq.reshape(2, 128).T
    bk2 = bk.reshape(2, 128).T
    bo_rep = np.broadcast_to((bv @ Wo32 + bo).reshape(1, 256), (128, 256))

    kv = [np.concatenate([key[b], value[b]], axis=1).astype(np.float16)
          for b in range(2)]
    qt2 = []
    for b in range(2):
        row = []
        for qi in range(4):
            qT = query[b, qi * SQ:(qi + 1) * SQ].T.astype(np.float16)
            row.append(np.ascontiguousarray(np.concatenate(
                [qT[128 * t:128 * (t + 1), 512 * sg:512 * (sg + 1)]
                 for sg in range(2) for t in range(2)], axis=1)))
        qt2.append(row)

    # host-side compaction + transpose: kvt5 [128, 4, skc] per batch
    kvt = []
    cfs = []
    for b in range(2):
        ix = idxs[b]
        nk = len(ix)
        ix_pad = np.concatenate(
            [ix, np.full(skc - nk, ix[0] if nk else 0, np.int32)])
        kvT = kv[b][ix_pad].T  # [512, skc]
        kvt.append(np.ascontiguousarray(kvT.reshape(4, 128, skc)
                                        .transpose(1, 0, 2)))
        mcomp = np.where(np.arange(skc) < nk, 0.0, NEG).astype(np.float32)
        mb = mcomp.reshape(nkc, 128).T
        cf = np.zeros((128, 516 + nkc), np.float32)
        cf[:, 0:2] = bq2
        cf[:, 2:4] = bk2
        cf[:, 4:260] = bo_rep
        cf[:, 260:260 + nkc] = mb
        cfs.append(np.ascontiguousarray(cf))

    in_maps = []
    for c in range(8):
        b, qi = divmod(c, 4)
        in_maps.append({
            "qt_in": qt2[b][qi],
            "kvt_in": kvt[b],
            "ch_in": ch,
            "cf_in": cfs[b],
        })

    res = run_bass_kernel_spmd(nc, in_maps, core_ids=list(range(8)),
                               trace=_trace)
    if _trace:
        _CACHE["last_result"] = res

    out = np.empty((2, 4096, HID), np.float32)
    for c in range(8):
        b, qi = divmod(c, 4)
        out[b, qi * SQ:(qi + 1) * SQ] = res.results[c]["out"]
    return out
